# revision 18
# baseline (speedup 1.0000x reference)
"""BailingMoE block on 8 Trainium2 NeuronCores.

Design (v2):
  - Attention: data-parallel tokens (core c owns tokens [128c,128c+128)).
    k/v are computed REPLICATED on every core (fp8 DoubleRow matmuls from a
    packed x^T) so no kv AllGather is needed. q/scores/ctx/wo for own chunk.
    All 16-bit attention data is FP16 (not bf16) to keep x2 noise low enough
    that router top-2 decisions never flip vs the fp32 reference.
  - rsq (rmsnorm row scales of x) via a tiny early 8-core AllGather.
  - MoE: 4 token-pair groups x 2 expert-halves. Core c = (pair c//2,
    half c%2) computes its 4 experts over the pair's 256 tokens.
    Dispatch: pair AllGather of h2^T as fp8 MAIN + fp8 RESIDUAL planes
    (~fp16 fidelity at fp8 DoubleRow speed; K-chain runs both) + fp32
    router weights (bitcast into the fp8 payload).
    Shared expert: half the IS columns per core, over the pair tokens, in
    FP16, folded into the same down-proj psum accumulation.
    Combine: pair ReduceScatter (fp16) of routed+shared partials.
  - Big matmuls fp8 DoubleRow (157 TMAC/s); fp16 elsewhere; fp32 psum,
    residual, router.
"""

import numpy as np

import concourse.bass as bass
import concourse.bacc as bacc
import concourse.mybir as mybir
import concourse.tile as tile
from concourse.bass_utils import run_bass_kernel_spmd
from concourse.masks import make_identity

F32 = mybir.dt.float32
F16 = mybir.dt.float16
F8 = mybir.dt.float8e4
AF = mybir.ActivationFunctionType
ALU = mybir.AluOpType
AX = mybir.AxisListType
PM = mybir.MatmulPerfMode

N_CORES = 8
T = 1024
TC = 128          # own tokens
TP = 256          # pair tokens
H = 2048
NH = 16
NKV = 4
DH = 128
E = 8
EH = 4            # experts per core
I = 1024
IS = 1024
ISH = IS // 2     # shared cols per core
KH = 16           # 128-tiles over H
KJ = 8            # 256-pair-tiles over H
EPS = 1e-6
SCALE = DH ** -0.5
NEG = -30000.0
EBIAS = -3.0

AS_H = 16.0
WS_GU = 1024.0
SILU_SC = 1.0 / (AS_H * WS_GU)
AS_A = 8.0
WS_D = 1024.0
DQ_D = 1.0 / (AS_A * WS_D)
U_SC = AS_A / (AS_H * WS_GU)
SHD_SC = AS_A * WS_D

RG_ALL = [list(range(N_CORES))]
RG_PAIR = [[0, 1], [2, 3], [4, 5], [6, 7]]

# ag payload (fp8 elements): [h8: KH*128*TC][r8: KH*128*TC][w: TC*E*4 bytes]
H8SZ = KH * 128 * TC
WOFF = 2 * H8SZ
AGSZ = WOFF + TC * E * 4

_cache = {}


def _bc(ap, n, axis=1):
    a = [list(p) for p in ap.ap]
    a.insert(axis, [0, n])
    return bass.AP(tensor=ap.tensor, offset=ap.offset, ap=a)


def build_nc():
    nc = bacc.Bacc("TRN2", target_bir_lowering=False, num_devices=N_CORES)

    x_res = nc.dram_tensor("x_res", [TC, H], F32, kind="ExternalInput")
    xT_in = nc.dram_tensor("xT_in", [KH, 128, T], F16, kind="ExternalInput")
    xTq_in = nc.dram_tensor("xTq_in", [KH, 128, TC], F16,
                            kind="ExternalInput")
    wkv_in = nc.dram_tensor("wkv_in", [KH, 128, 2 * NKV * DH], F16,
                            kind="ExternalInput")
    wq_in = nc.dram_tensor("wq_in", [4, KH, 128, 512], F16,
                           kind="ExternalInput")
    wo_in = nc.dram_tensor("wo_in", [4, KH, 128, 512], F16,
                           kind="ExternalInput")
    rope_k = nc.dram_tensor("rope_k", [8, TC, 4, 64], F16,
                            kind="ExternalInput")
    rope_q = nc.dram_tensor("rope_q", [TC, 4, 64], F16, kind="ExternalInput")
    mask_in = nc.dram_tensor("mask_in", [128, 8, TC], F16,
                             kind="ExternalInput")
    wrT = nc.dram_tensor("wrT", [128, KH, E], F32, kind="ExternalInput")
    wsgu = nc.dram_tensor("wsgu", [KH, 128, 2 * ISH], F16,
                          kind="ExternalInput")
    wsd = nc.dram_tensor("wsd", [ISH // 128, 128, H], F16,
                         kind="ExternalInput")
    wgu_in = nc.dram_tensor("wgu_in", [EH, 16, 128, KJ, 2, 128], F8,
                            kind="ExternalInput")
    wd_in = nc.dram_tensor("wd_in", [EH, 4, 128, 2, H], F8,
                           kind="ExternalInput")
    esel = nc.dram_tensor("esel", [1, EH * E], F32, kind="ExternalInput")
    out_chunk = nc.dram_tensor("out_chunk", [TC, H], F32,
                               kind="ExternalOutput")
    dbg_x2 = nc.dram_tensor("dbg_x2", [TC, H], F32, kind="ExternalOutput")
    dbg_wm = nc.dram_tensor("dbg_wm", [TC, E], F32, kind="ExternalOutput")
    dbg_moe = nc.dram_tensor("dbg_moe", [TC, H], F16, kind="ExternalOutput")

    with tile.TileContext(nc) as tc:
        with tc.tile_pool(name="dram", bufs=1, space="DRAM") as dram, \
             tc.tile_pool(name="const", bufs=1) as const, \
             tc.tile_pool(name="mid", bufs=1) as mid, \
             tc.tile_pool(name="psA", bufs=2, space="PSUM") as psA, \
             tc.tile_pool(name="psB", bufs=2, space="PSUM") as psB, \
             tc.tile_pool(name="pstr", bufs=2, space="PSUM") as pstr:

            # ---- DRAM collective buffers ----
            rsq_in = dram.tile([TC], F32)
            rsq_out = dram.tile([T], F32, addr_space="Shared")
            ag_in = dram.tile([AGSZ], F8)
            ag_out = dram.tile([2 * AGSZ], F8)
            rs_in = dram.tile([TP, H], F16)
            rs_out = dram.tile([TC, H], F16)
            wrow_dram = dram.tile([EH, TP], F32)

            # ---- constants ----
            ident16 = const.tile([128, 128], F16)
            make_identity(nc, ident16)
            ident32 = const.tile([128, 128], F32)
            make_identity(nc, ident32)
            eps_sb = const.tile([128, 1], F32)
            nc.vector.memset(eps_sb, EPS)
            neg3 = const.tile([128, 1], F32)
            nc.vector.memset(neg3, EBIAS)
            esel_sb = const.tile([128, EH * E], F32)
            nc.gpsimd.dma_start(
                out=esel_sb,
                in_=bass.AP(tensor=esel, offset=0, ap=[[0, 128], [1, EH * E]]))
            mask_sb = const.tile([128, 8, TC], F16)
            nc.gpsimd.dma_start(out=mask_sb, in_=mask_in[:, :, :])
            wrT_sb = const.tile([128, KH, E], F32)
            nc.gpsimd.dma_start(out=wrT_sb, in_=wrT[:, :, :])

            # ---- persistent tiles (whole kernel) ----
            x_sb = mid.tile([TC, H], F32)
            x2_sb = mid.tile([TC, H], F32)
            rsqv = mid.tile([128, 8], F32)

            # ================= ATTENTION (scoped pool) =================
            with tc.tile_pool(name="att", bufs=2) as att:
                # --- early loads ---
                nc.sync.dma_start(out=x_sb, in_=x_res[:, :])
                xT_sb = att.tile([128, KH, T], F16, tag="xT", bufs=1)
                wkv_sb = att.tile([128, KH, 1024], F16, tag="wkv", bufs=1)
                for j in range(KH):
                    nc.sync.dma_start(
                        out=xT_sb[:, j, :],
                        in_=bass.AP(tensor=xT_in, offset=j * 128 * T,
                                    ap=[[T, 128], [1, T]]))
                    nc.scalar.dma_start(
                        out=wkv_sb[:, j, :],
                        in_=bass.AP(tensor=wkv_in, offset=j * 128 * 1024,
                                    ap=[[1024, 128], [1, 1024]]))
                xTq_sb = att.tile([128, KH, TC], F16, tag="xTq", bufs=1)
                nc.gpsimd.dma_start(
                    out=xTq_sb,
                    in_=bass.AP(tensor=xTq_in, offset=0,
                                ap=[[TC, 128], [128 * TC, KH], [1, TC]]))
                rope_k_sb = att.tile([128, 8, 4, 64], F16, tag="ropek",
                                     bufs=1)
                nc.gpsimd.dma_start(
                    out=rope_k_sb,
                    in_=bass.AP(tensor=rope_k, offset=0,
                                ap=[[256, 128], [TC * 256, 8], [64, 4],
                                    [1, 64]]))
                rope_q_sb = att.tile([TC, 4, 64], F16, tag="ropeq", bufs=1)
                nc.gpsimd.dma_start(out=rope_q_sb, in_=rope_q[:, :, :])

                # rsq of own chunk -> 8-core AllGather (collective idle now)
                red4 = att.tile([TC, 4], F32, tag="red4", bufs=1)
                for n in range(4):
                    sl = slice(n * 512, (n + 1) * 512)
                    sqp = att.tile([TC, 512], F32, tag="sqp")
                    nc.vector.tensor_mul(sqp, x_sb[:, sl], x_sb[:, sl])
                    nc.vector.tensor_reduce(red4[:, n:n + 1], sqp, axis=AX.X,
                                            op=ALU.add)
                rsqo = att.tile([TC, 1], F32, tag="rsqo", bufs=1)
                nc.vector.tensor_reduce(rsqo, red4, axis=AX.X, op=ALU.add)
                nc.scalar.activation(rsqo, rsqo, AF.Sqrt, bias=eps_sb[:TC],
                                     scale=1.0 / H)
                nc.vector.reciprocal(rsqo, rsqo)
                nc.gpsimd.dma_start(
                    out=bass.AP(tensor=rsq_in.tensor, offset=rsq_in.offset,
                                ap=[[1, TC]]),
                    in_=rsqo.rearrange("p one -> p (one)"))
                nc.gpsimd.collective_compute(
                    "AllGather", ALU.bypass, replica_groups=RG_ALL,
                    ins=[rsq_in.opt()], outs=[rsq_out.opt()])

                # rsq arrives: per-chunk v scale
                nc.gpsimd.dma_start(
                    out=rsqv,
                    in_=bass.AP(tensor=rsq_out.tensor, offset=rsq_out.offset,
                                ap=[[1, 128], [128, 8]]))
    

                kT_all = att.tile([128, NKV, 8, TC], F16, tag="kT", bufs=1)
                v_all = att.tile([128, 8, NKV, DH + 1], F16, tag="v", bufs=1)
                qT_all = att.tile([128, NH, TC], F16, tag="qT", bufs=1)
                ctxT_all = att.tile([128, NH, TC], F16, tag="ctxT", bufs=1)
                h2_f16 = att.tile([TC, H], F16, tag="h2f16", bufs=1)
                h8_sb = att.tile([128, KH, TC], F8, tag="h8", bufs=1)
                r8_sb = att.tile([128, KH, TC], F8, tag="r8", bufs=1)

                def rope(x3, o3, nh, tab):
                    c1 = _bc(tab[:, 0, :], nh)
                    s1 = _bc(tab[:, 1, :], nh)
                    c2 = _bc(tab[:, 2, :], nh)
                    s2 = _bc(tab[:, 3, :], nh)
                    x1 = x3[:, :, 0:64]
                    x2_ = x3[:, :, 64:128]
                    t1 = att.tile([TC, NKV, 64], F32, tag="rp1")
                    tn = att.tile([TC, NKV, 64], F32, tag="rpn")
                    t1v = t1[:, :nh, :]
                    tnv = tn[:, :nh, :]
                    nc.vector.tensor_mul(t1v, x1, c1)
                    nc.gpsimd.tensor_mul(tnv, x2_, s1)
                    nc.vector.tensor_sub(o3[:, :, 0:64], t1v, tnv)
                    nc.gpsimd.tensor_mul(t1v, x2_, c2)
                    nc.vector.tensor_mul(tnv, x1, s2)
                    nc.gpsimd.tensor_add(o3[:, :, 64:128], t1v, tnv)

                def qk_norm(raw3, nh, tag):
                    # raw3 [TC, nh<=4, DH] f32, normed in place
                    sq = att.tile([TC, NKV, DH], F32, tag="nsq")
                    sqv = sq[:, :nh, :]
                    nc.vector.tensor_mul(sqv, raw3, raw3)
                    red = att.tile([TC, NKV, 1], F32, tag=f"nred{tag}")
                    nc.vector.tensor_reduce(red[:, :nh, :], sqv, axis=AX.X,
                                            op=ALU.add)
                    redf = red[:, :nh, :].rearrange("p h one -> p (h one)")
                    nc.scalar.activation(redf, redf, AF.Sqrt,
                                         bias=eps_sb[:TC], scale=1.0 / DH)
                    nc.vector.reciprocal(redf, redf)
                    for h in range(nh):
                        nc.vector.tensor_scalar_mul(
                            raw3[:, h, :], raw3[:, h, :], red[:, h, :])

                # --- k/v for all 8 chunks (replicated) ---
                for c in range(8):
                    pkv = psA.tile([128, 1024], F32, tag="p1k")
                    for hf in range(2):
                        for j in range(KH):
                            nc.tensor.matmul(
                                pkv[:, hf * 512:(hf + 1) * 512],
                                xT_sb[:, j, c * TC:(c + 1) * TC],
                                wkv_sb[:, j, hf * 512:(hf + 1) * 512],
                                start=(j == 0), stop=(j == KH - 1))
                    kraw = att.tile([TC, 512], F32, tag="kraw")
                    nc.scalar.activation(kraw, pkv[:, 0:512], AF.Copy,
                                         scale=1.0)
                    k3 = kraw.rearrange("p (h d) -> p h d", h=NKV)
                    qk_norm(k3, NKV, "k")
                    kf = att.tile([TC, NKV, DH], F16, tag="kf")
                    rope(k3, kf, NKV, rope_k_sb[:, c, :, :])
                    nc.vector.tensor_scalar_mul(
                        v_all[:, c, :, 0:DH],
                        pkv[:, 512:1024].rearrange("p (g d) -> p g d", g=NKV),
                        rsqv[:, c:c + 1])
                    nc.gpsimd.memset(v_all[:, c, :, DH:DH + 1], 1.0)
                    for g in range(NKV):
                        pt = pstr.tile([128, 128], F16, tag="pt")
                        nc.tensor.transpose(pt, kf[:, g, :], ident16)
                        if g % 2 == 0:
                            nc.vector.tensor_copy(kT_all[:, g, c, :], pt)
                        else:
                            nc.scalar.activation(kT_all[:, g, c, :], pt,
                                                 AF.Copy)

                # --- q for own chunk, 4 head-groups of 4 ---
                for ng in range(4):
                    wqs = att.tile([128, KH, 512], F16, tag="wst")
                    nc.scalar.dma_start(
                        out=wqs,
                        in_=bass.AP(tensor=wq_in,
                                    offset=ng * KH * 128 * 512,
                                    ap=[[512, 128], [128 * 512, KH],
                                        [1, 512]]))
                    pq = psB.tile([TC, 512], F32, tag="p512")
                    for j in range(KH):
                        nc.tensor.matmul(
                            pq, xTq_sb[:, j, :], wqs[:, j, :],
                            start=(j == 0), stop=(j == KH - 1))
                    qraw = att.tile([TC, 512], F32, tag="qraw")
                    nc.scalar.activation(qraw, pq, AF.Copy, scale=1.0)
                    q3 = qraw.rearrange("p (h d) -> p h d", h=4)
                    qk_norm(q3, 4, "q")
                    qf = att.tile([TC, 4, DH], F16, tag="qf")
                    rope(q3, qf, 4, rope_q_sb)
                    for hh in range(4):
                        pt = pstr.tile([128, 128], F16, tag="pt")
                        nc.tensor.transpose(pt, qf[:, hh, :], ident16)
                        nc.vector.tensor_copy(qT_all[:, ng * 4 + hh, :], pt)

                # --- per-head scores / ctx (1-deep software pipeline) ---
                def head_front(h):
                    g = h // (NH // NKV)
                    ps = psA.tile([128, 1024], F32, tag="p1k")
                    for c8 in range(8):
                        nc.tensor.matmul(
                            ps[:, c8 * TC:(c8 + 1) * TC],
                            kT_all[:, g, c8, :], qT_all[:, h, :],
                            start=True, stop=True)
                    mf = mask_sb.rearrange("p c q -> p (c q)")
                    nc.vector.tensor_add(ps, ps, mf)
                    pr = att.tile([128, 8, TC], F16, tag="probs")
                    nc.scalar.activation(
                        pr.rearrange("p c q -> p (c q)"), ps, AF.Exp,
                        bias=neg3, scale=SCALE)
                    return pr

                def head_tail(h, pr):
                    g = h // (NH // NKV)
                    pct = psB.tile([TC, 512], F32, tag="p512")
                    pc = pct[:, 0:DH + 1]
                    for c8 in range(8):
                        nc.tensor.matmul(pc, pr[:, c8, :], v_all[:, c8, g, :],
                                         start=(c8 == 0), stop=(c8 == 7))
                    rden = att.tile([TC, 1], F32, tag="rden")
                    nc.vector.reciprocal(rden, pc[:, DH:DH + 1])
                    cf = att.tile([TC, DH], F16, tag="cf")
                    nc.vector.tensor_scalar_mul(cf, pc[:, 0:DH], rden)
                    pt = pstr.tile([128, 128], F16, tag="pt")
                    nc.tensor.transpose(pt, cf, ident16)
                    nc.scalar.activation(ctxT_all[:, h, :], pt,
                                         AF.Copy, scale=1.0)

                pend = None
                for h in range(NH):
                    pr = head_front(h)
                    if pend is not None:
                        head_tail(*pend)
                    pend = (h, pr)
                head_tail(*pend)

                # --- wo + residual + rms stats ---
                for n in range(4):
                    wos = att.tile([128, KH, 512], F16, tag="wst")
                    nc.sync.dma_start(
                        out=wos,
                        in_=bass.AP(tensor=wo_in,
                                    offset=n * KH * 128 * 512,
                                    ap=[[512, 128], [128 * 512, KH],
                                        [1, 512]]))
                    po = psB.tile([TC, 512], F32, tag="p512")
                    for j in range(KH):
                        nc.tensor.matmul(
                            po, ctxT_all[:, j, :], wos[:, j, :],
                            start=(j == 0), stop=(j == KH - 1))
                    at16 = att.tile([TC, 512], F16, tag="at16")
                    nc.scalar.activation(at16, po, AF.Copy, scale=1.0)
                    sl = slice(n * 512, (n + 1) * 512)
                    nc.vector.tensor_add(x2_sb[:, sl], at16, x_sb[:, sl])
                    sqp = att.tile([TC, 512], F32, tag="sqp")
                    nc.gpsimd.tensor_mul(sqp, x2_sb[:, sl], x2_sb[:, sl])
                    nc.vector.tensor_reduce(red4[:, n:n + 1], sqp, axis=AX.X,
                                            op=ALU.add)

                # rs2 true + rs2*AS_H
                rs2t = att.tile([TC, 1], F32, tag="rs2t", bufs=1)
                nc.vector.tensor_reduce(rs2t, red4, axis=AX.X, op=ALU.add)
                nc.scalar.activation(rs2t, rs2t, AF.Sqrt, bias=eps_sb[:TC],
                                     scale=1.0 / H)
                nc.vector.reciprocal(rs2t, rs2t)
                rs2h = att.tile([TC, 1], F32, tag="rs2h", bufs=1)
                nc.scalar.activation(rs2h, rs2t, AF.Copy, scale=AS_H)
                nc.vector.tensor_scalar_mul(h2_f16, x2_sb, rs2h)

                # --- router (fp32, flip-safe) ---
                prlt = psA.tile([128, 1024], F32, tag="p1k")
                prl = prlt[:, 0:E]
                for j in range(KH):
                    ptft = psB.tile([TC, 512], F32, tag="p512")
                    ptf = ptft[:, 0:128]
                    nc.tensor.transpose(
                        ptf, x2_sb[:, j * 128:(j + 1) * 128], ident32)
                    x2T = att.tile([128, TC], F32, tag="x2T")
                    nc.vector.tensor_copy(x2T, ptf)
                    nc.tensor.matmul(prl, x2T, wrT_sb[:, j, :],
                                     start=(j == 0), stop=(j == KH - 1))
                lg = att.tile([TC, E], F32, tag="lg", bufs=1)
                nc.vector.tensor_scalar_mul(lg, prl, rs2t)
                probs8 = att.tile([TC, E], F32, tag="probs8", bufs=1)
                nc.scalar.activation(probs8, lg, AF.Exp, scale=1.0)
                den8 = att.tile([TC, 1], F32, tag="den8", bufs=1)
                nc.vector.tensor_reduce(den8, probs8, axis=AX.X, op=ALU.add)
                nc.vector.reciprocal(den8, den8)
                nc.vector.tensor_scalar_mul(probs8, probs8, den8)
                mx8 = att.tile([TC, 8], F32, tag="mx8", bufs=1)
                nc.vector.max(out=mx8, in_=probs8)
                s12 = att.tile([TC, 1], F32, tag="s12", bufs=1)
                nc.vector.tensor_add(s12, mx8[:, 0:1], mx8[:, 1:2])
                nc.vector.reciprocal(s12, s12)
                eq1 = att.tile([TC, E], F32, tag="eq1", bufs=1)
                nc.vector.tensor_scalar(eq1, probs8, mx8[:, 0:1], None,
                                        op0=ALU.is_equal)
                eq2 = att.tile([TC, E], F32, tag="eq2", bufs=1)
                nc.vector.tensor_scalar(eq2, probs8, mx8[:, 1:2], None,
                                        op0=ALU.is_equal)
                nc.vector.tensor_add(eq1, eq1, eq2)
                wm = att.tile([TC, E], F32, tag="wm", bufs=1)
                nc.vector.tensor_mul(wm, probs8, eq1)
                nc.vector.tensor_scalar_mul(wm, wm, s12)
                nc.gpsimd.dma_start(
                    out=bass.AP(tensor=ag_in.tensor,
                                offset=ag_in.offset + WOFF,
                                ap=[[E * 4, TC], [1, E * 4]]),
                    in_=wm.bitcast(F8))
                nc.gpsimd.dma_start(out=dbg_wm[:, :], in_=wm)
                nc.gpsimd.dma_start(out=dbg_x2[:, :], in_=x2_sb)

                # --- h2 -> h8 + r8 planes ---
                for j in range(KH):
                    pt = pstr.tile([128, 128], F16, tag="pt")
                    nc.tensor.transpose(
                        pt, h2_f16[:, j * 128:(j + 1) * 128], ident16)
                    nc.scalar.activation(h8_sb[:, j, :], pt, AF.Copy,
                                         scale=1.0)
                    dq16 = att.tile([128, TC], F16, tag="dq16")
                    nc.scalar.activation(dq16, h8_sb[:, j, :], AF.Copy,
                                         scale=1.0)
                    rr16 = att.tile([128, TC], F16, tag="rr16")
                    nc.vector.tensor_sub(rr16, pt, dq16)
                    nc.scalar.activation(r8_sb[:, j, :], rr16, AF.Copy,
                                         scale=1.0)
                nc.sync.dma_start(
                    out=bass.AP(tensor=ag_in.tensor, offset=ag_in.offset,
                                ap=[[TC, 128], [128 * TC, KH], [1, TC]]),
                    in_=h8_sb)
                nc.scalar.dma_start(
                    out=bass.AP(tensor=ag_in.tensor,
                                offset=ag_in.offset + H8SZ,
                                ap=[[TC, 128], [128 * TC, KH], [1, TC]]),
                    in_=r8_sb)

            nc.gpsimd.collective_compute(
                "AllGather", ALU.bypass, replica_groups=RG_PAIR,
                ins=[ag_in.opt()], outs=[ag_out.opt()])

            # ================= MOE =================
            with tc.tile_pool(name="moe", bufs=2) as moe, \
                 tc.tile_pool(name="wgup", bufs=3) as wgup:

                # weights loaded in the AG window (DMA is idle there)
                wsgu_sb = moe.tile([128, KH, 2 * ISH], F16, tag="wsgu",
                                   bufs=1)
                nc.scalar.dma_start(
                    out=wsgu_sb,
                    in_=bass.AP(tensor=wsgu, offset=0,
                                ap=[[2 * ISH, 128], [128 * 2 * ISH, KH],
                                    [1, 2 * ISH]]))
                wsd_sb = moe.tile([128, ISH // 128, H], F16, tag="wsd",
                                  bufs=1)
                nc.sync.dma_start(
                    out=wsd_sb,
                    in_=bass.AP(tensor=wsd, offset=0,
                                ap=[[H, 128], [128 * H, ISH // 128],
                                    [1, H]]))
                wd_sb = moe.tile([128, EH * 4, 2, H], F8, tag="wd", bufs=1)
                for s in range(EH):
                    (nc.sync if s % 2 == 0 else nc.scalar).dma_start(
                        out=wd_sb[:, s * 4:(s + 1) * 4, :, :],
                        in_=bass.AP(tensor=wd_in, offset=s * 4 * 128 * 2 * H,
                                    ap=[[2 * H, 128], [128 * 2 * H, 4],
                                        [H, 2], [1, H]]))

                # gathered h8/r8 pair planes [128, jj, pl, tok(rank-major)]
                h8P = moe.tile([128, KJ, 2, TP], F8, tag="h8P", bufs=1)
                r8P = moe.tile([128, KJ, 2, TP], F8, tag="r8P", bufs=1)
                for r in range(2):
                    base = ag_out.offset + r * AGSZ
                    nc.gpsimd.dma_start(
                        out=h8P[:, :, :, r * TC:(r + 1) * TC],
                        in_=bass.AP(tensor=ag_out.tensor, offset=base,
                                    ap=[[TC, 128], [2 * 128 * TC, KJ],
                                        [128 * TC, 2], [1, TC]]))
                    nc.gpsimd.dma_start(
                        out=r8P[:, :, :, r * TC:(r + 1) * TC],
                        in_=bass.AP(tensor=ag_out.tensor, offset=base + H8SZ,
                                    ap=[[TC, 128], [2 * 128 * TC, KJ],
                                        [128 * TC, 2], [1, TC]]))
                # router weights of the pair
                wpair = moe.tile([128, 2, E], F32, tag="wpair", bufs=1)
                for r in range(2):
                    nc.gpsimd.dma_start(
                        out=wpair[:, r, :],
                        in_=bass.AP(tensor=ag_out.tensor,
                                    offset=ag_out.offset + r * AGSZ + WOFF,
                                    ap=[[E * 4, TC], [1, E * 4]]).bitcast(F32))

                # reconstruct fp16 h2 pair (for shared expert), 2 half passes
                h2P16 = moe.tile([128, KJ, 2, TP], F16, tag="h2P16", bufs=1)
                for hv in range(2):
                    hs = slice(hv * (KJ // 2), (hv + 1) * (KJ // 2))
                    tmp16 = moe.tile([128, KJ // 2, 2, TP], F16, tag="tmp16")
                    nc.scalar.activation(
                        h2P16[:, hs, :, :].rearrange("p a b t -> p (a b t)"),
                        h8P[:, hs, :, :].rearrange("p a b t -> p (a b t)"),
                        AF.Copy, scale=1.0)
                    nc.scalar.activation(
                        tmp16.rearrange("p a b t -> p (a b t)"),
                        r8P[:, hs, :, :].rearrange("p a b t -> p (a b t)"),
                        AF.Copy, scale=1.0)
                    nc.vector.tensor_add(
                        h2P16[:, hs, :, :].rearrange("p a b t -> p (a b t)"),
                        h2P16[:, hs, :, :].rearrange("p a b t -> p (a b t)"),
                        tmp16.rearrange("p a b t -> p (a b t)"))

                # per-expert token weight rows -> broadcast tiles
                wcols = moe.tile([128, 2, EH], F32, tag="wcols", bufs=1)
                for s in range(EH):
                    wtmp = moe.tile([128, 2, E], F32, tag="wtmp")
                    nc.vector.tensor_mul(
                        wtmp, wpair,
                        _bc(esel_sb[:, s * E:(s + 1) * E], 2))
                    nc.vector.tensor_reduce(wcols[:, :, s:s + 1], wtmp,
                                            axis=AX.X, op=ALU.add)
                for r in range(2):
                    nc.gpsimd.dma_start(
                        out=bass.AP(tensor=wrow_dram.tensor,
                                    offset=wrow_dram.offset + r * TC,
                                    ap=[[1, 128], [TP, EH]]),
                        in_=wcols[:, r, :])
                w_bcast = moe.tile([128, EH, TP], F32, tag="w_bcast", bufs=1)
                for s in range(EH):
                    nc.gpsimd.dma_start(
                        out=w_bcast[:, s, :],
                        in_=bass.AP(tensor=wrow_dram.tensor,
                                    offset=wrow_dram.offset + s * TP,
                                    ap=[[0, 128], [1, TP]]))

                # --- shared expert (fp16, half IS, pair tokens) ---
                actsT_sh = moe.tile([128, ISH // 128, TP], F16, tag="actsTsh",
                                    bufs=1)
                for ch in range(2):
                    psg = psA.tile([128, 1024], F32, tag="p1k")
                    for hf in range(2):
                        for j16 in range(KH):
                            nc.tensor.matmul(
                                psg[:, hf * 512:(hf + 1) * 512],
                                h2P16[:, j16 // 2, j16 % 2,
                                      ch * TC:(ch + 1) * TC],
                                wsgu_sb[:, j16, hf * 512:(hf + 1) * 512],
                                start=(j16 == 0), stop=(j16 == KH - 1))
                    gsh = moe.tile([128, ISH], F16, tag="gsh")
                    nc.scalar.activation(gsh, psg[:, 0:ISH], AF.Silu,
                                         scale=1.0 / AS_H)
                    ush = moe.tile([128, ISH], F16, tag="ush")
                    nc.scalar.activation(ush, psg[:, ISH:2 * ISH], AF.Copy,
                                         scale=1.0 / AS_H)
                    acts = moe.tile([128, ISH], F16, tag="acts")
                    nc.vector.tensor_mul(acts, gsh, ush)
                    for it in range(ISH // 128):
                        pt = pstr.tile([128, 128], F16, tag="pt")
                        nc.tensor.transpose(
                            pt, acts[:, it * 128:(it + 1) * 128], ident16)
                        if it % 2 == 0:
                            nc.vector.tensor_copy(
                                actsT_sh[:, it, ch * TC:(ch + 1) * TC], pt)
                        else:
                            nc.scalar.activation(
                                actsT_sh[:, it, ch * TC:(ch + 1) * TC], pt,
                                AF.Copy)

                # --- routed experts: gu (fp8 DR, h8+r8 K-chain) ---
                act_pr = moe.tile([128, EH * 4, 2, TP], F8, tag="act_pr",
                                  bufs=1)
                for s in range(EH):
                    g16 = moe.tile([128, 8, TP], F16, tag="g16")
                    wkc = None
                    for mi in range(16):
                        if mi % 2 == 0:
                            wkc = wgup.tile([128, 2, KJ, 2, 128], F8,
                                            tag="wgu")
                            (nc.sync if mi % 4 == 0 else nc.scalar).dma_start(
                                out=wkc,
                                in_=bass.AP(
                                    tensor=wgu_in,
                                    offset=(s * 16 + mi) * 128 * 2048,
                                    ap=[[2048, 128], [128 * 2048, 2],
                                        [256, KJ], [128, 2], [1, 128]]))
                        pgt = psB.tile([TC, 512], F32, tag="p512")
                        pg = pgt[:, 0:TP]
                        for jj in range(KJ):
                            nc.tensor.matmul(
                                pg, wkc[:, mi % 2, jj, :, :],
                                h8P[:, jj, :, :],
                                start=(jj == 0), stop=False,
                                perf_mode=PM.DoubleRow)
                        for jj in range(KJ):
                            nc.tensor.matmul(
                                pg, wkc[:, mi % 2, jj, :, :],
                                r8P[:, jj, :, :],
                                start=False, stop=(jj == KJ - 1),
                                perf_mode=PM.DoubleRow)
                        if mi < 8:
                            nc.scalar.activation(g16[:, mi, :], pg, AF.Silu,
                                                 scale=SILU_SC)
                        else:
                            iu = mi - 8
                            u16 = moe.tile([128, TP], F16, tag="u16", bufs=3)
                            nc.vector.tensor_mul(u16, pg, w_bcast[:, s, :])
                            (nc.vector if iu % 2 == 0
                             else nc.gpsimd).tensor_mul(
                                act_pr[:, s * 4 + iu // 2, iu % 2, :],
                                g16[:, iu, :], u16)

                # --- down proj (routed fp8 DR + shared fp16 in one psum) ---
                for ch in range(2):
                    for n in range(4):
                        pd = psB.tile([TC, 512], F32, tag="p512")
                        for kk in range(EH * 4):
                            nc.tensor.matmul(
                                pd,
                                act_pr[:, kk, :, ch * TC:(ch + 1) * TC],
                                wd_sb[:, kk, :, n * 512:(n + 1) * 512],
                                start=(kk == 0), stop=False,
                                perf_mode=PM.DoubleRow)
                        for it in range(ISH // 128):
                            nc.tensor.matmul(
                                pd,
                                actsT_sh[:, it, ch * TC:(ch + 1) * TC],
                                wsd_sb[:, it, n * 512:(n + 1) * 512],
                                start=False, stop=(it == ISH // 128 - 1))
                        rsd = moe.tile([TC, 512], F16, tag="rsd", bufs=3)
                        nc.scalar.activation(rsd, pd, AF.Copy, scale=DQ_D)
                        (nc.sync if (ch * 4 + n) % 2 == 0
                         else nc.scalar).dma_start(
                            out=rs_in[ch * TC:(ch + 1) * TC,
                                      n * 512:(n + 1) * 512],
                            in_=rsd)

                nc.gpsimd.collective_compute(
                    "ReduceScatter", ALU.add, replica_groups=RG_PAIR,
                    ins=[rs_in.opt()], outs=[rs_out.opt()])

                # --- combine + residual ---
                for q in range(4):
                    sl = slice(q * 512, (q + 1) * 512)
                    rsld = moe.tile([TC, 512], F16, tag="rsld")
                    (nc.sync if q % 2 == 0 else nc.scalar).dma_start(
                        out=rsld, in_=rs_out[:, sl])
                    outf = moe.tile([TC, 512], F32, tag="outf")
                    nc.vector.tensor_add(outf, x2_sb[:, sl], rsld)
                    (nc.sync if q % 2 == 0 else nc.scalar).dma_start(
                        out=out_chunk[:, sl], in_=outf)
                    nc.gpsimd.dma_start(out=dbg_moe[:, sl], in_=rsld)

    nc.compile()
    return nc


def _prep_inputs(hidden_states, w_ln1, w_ln2, wqkv, q_norm_w, k_norm_w, wo,
                 w_router, w_gu, w_d, ws_gu, ws_d, positions):
    import ml_dtypes
    f16 = np.float16
    f8 = ml_dtypes.float8_e4m3

    x = np.asarray(hidden_states, np.float32).reshape(T, H)
    w_ln1 = np.asarray(w_ln1, np.float32)
    w_ln2 = np.asarray(w_ln2, np.float32)
    wqkv_e = np.asarray(wqkv, np.float32) * w_ln1[:, None]  # [H, 3072]

    def pack_dr(W, scale):
        # W [Hrows, C] -> [KJ, 128, 2, C]; row = 256*jj + 128*pl + r
        Wr = (np.asarray(W, np.float32) * scale).reshape(KJ, 2, 128, -1)
        return np.ascontiguousarray(Wr.transpose(0, 2, 1, 3)).astype(f8)

    def pack16(W):
        # W [Hrows, C] -> [KH, 128, C] fp16
        return np.ascontiguousarray(
            np.asarray(W, np.float32).reshape(KH, 128, -1)).astype(f16)

    def by_coltile16(W):
        # W [Hrows, C] -> [C//512, KH, 128, 512] fp16
        C = W.shape[1]
        return np.ascontiguousarray(
            np.asarray(W, np.float32).reshape(KH, 128, C // 512, 512)
            .transpose(2, 0, 1, 3)).astype(f16)

    wq_p = by_coltile16(wqkv_e[:, :NH * DH])
    wkv_p = pack16(wqkv_e[:, NH * DH:])
    xT_p = pack16(x.T)
    wo_p = by_coltile16(np.asarray(wo, np.float32))

    pos = np.asarray(positions).astype(np.float64)
    inv_freq = 1.0 / (10000.0 ** (np.arange(0, DH, 2, dtype=np.float64) / DH))
    freqs = pos[:, None] * inv_freq[None, :]
    cos = np.cos(freqs).astype(np.float32)
    sin = np.sin(freqs).astype(np.float32)
    qw = np.asarray(q_norm_w, np.float32)
    kw = np.asarray(k_norm_w, np.float32)

    def rope_tab(w):
        return np.ascontiguousarray(
            np.stack([cos * w[None, :64], sin * w[None, 64:],
                      cos * w[None, 64:], sin * w[None, :64]],
                     axis=1)).astype(f16)

    rq = rope_tab(qw)  # [T, 4, 64]
    rk = rope_tab(kw).reshape(8, TC, 4, 64)

    wrT_e = (np.asarray(w_router, np.float32) * w_ln2[None, :]).T  # [H, E]
    wrT_p = np.ascontiguousarray(
        wrT_e.reshape(KH, 128, E).transpose(1, 0, 2)).astype(np.float32)

    ws_gu_e = np.asarray(ws_gu, np.float32) * w_ln2[:, None]
    ws_d_e = np.asarray(ws_d, np.float32) * SHD_SC
    w_gu_e = np.asarray(w_gu, np.float32) * w_ln2[None, :, None] * WS_GU
    w_d_e = np.asarray(w_d, np.float32) * WS_D

    kidx = np.arange(T)
    in_maps = []
    for c in range(N_CORES):
        rows = np.arange(c * TC, (c + 1) * TC)
        # mask[p, kc, q]: k token = kc*128+p, q token = c*128+q
        kk = kidx.reshape(8, 128)
        mask = np.where(kk.T[:, :, None] <= rows[None, None, :], 0.0, NEG)
        mask = np.ascontiguousarray(mask).astype(f16)

        ph = c % 2
        # shared half: own g/u columns
        wsgu_half = np.concatenate(
            [ws_gu_e[:, ph * ISH:(ph + 1) * ISH],
             ws_gu_e[:, IS + ph * ISH:IS + (ph + 1) * ISH]], axis=1)
        wsgu_p = np.ascontiguousarray(
            wsgu_half.reshape(KH, 128, 2 * ISH)).astype(f16)
        wsd_p = np.ascontiguousarray(
            ws_d_e[ph * ISH:(ph + 1) * ISH].reshape(ISH // 128, 128, H)
        ).astype(f16)

        # routed experts for this half
        es = np.zeros((1, EH * E), np.float32)
        wgu_p = np.empty((EH, 16, 128, KJ, 2, 128), f8)
        wd_p = np.empty((EH, 4, 128, 2, H), f8)
        for s in range(EH):
            e = ph * EH + s
            es[0, s * E + e] = U_SC
            wg = w_gu_e[e].reshape(KJ, 2, 128, 16, 128)  # [jj, pl, r, mi, m]
            wgu_p[s] = wg.transpose(3, 2, 0, 1, 4).astype(f8)
            wdv = w_d_e[e].reshape(4, 2, 128, H)  # [j, pl, r, h]
            wd_p[s] = wdv.transpose(0, 2, 1, 3).astype(f8)

        in_maps.append({
            "x_res": np.ascontiguousarray(x[c * TC:(c + 1) * TC]),
            "xT_in": xT_p,
            "xTq_in": np.ascontiguousarray(
                xT_p[:, :, c * TC:(c + 1) * TC]),
            "wkv_in": wkv_p,
            "wq_in": wq_p,
            "wo_in": wo_p,
            "rope_k": rk,
            "rope_q": np.ascontiguousarray(rq[c * TC:(c + 1) * TC]),
            "mask_in": mask,
            "wrT": wrT_p,
            "wsgu": wsgu_p,
            "wsd": wsd_p,
            "wgu_in": np.ascontiguousarray(wgu_p),
            "wd_in": np.ascontiguousarray(wd_p),
            "esel": es,
        })
    return in_maps


def kernel(**inputs):
    import os
    if "nc" not in _cache:
        _cache["nc"] = build_nc()
    nc = _cache["nc"]
    in_maps = _prep_inputs(**inputs)
    trace = bool(int(os.environ.get("KERNEL_TRACE", "0")))
    res = run_bass_kernel_spmd(nc, in_maps, core_ids=list(range(N_CORES)),
                               trace=trace)
    _cache["last_result"] = res
    out = np.concatenate(
        [res.results[c]["out_chunk"] for c in range(N_CORES)], axis=0)
    return out.reshape(1, T, H).astype(np.float32)


if __name__ == "__main__":
    import reference
    inp = {k: np.asarray(v) for k, v in reference.setup_inputs().items()}
    got = kernel(**inp)
    exp = np.asarray(reference.reference(**reference.setup_inputs()))
    denom = np.abs(exp).max()
    err = np.abs(got - exp).max() / denom
    print("abs max:", denom, "rel err:", err)


# revision 21
# speedup vs baseline: 1.0371x; 1.0371x over previous
"""BailingMoE block on 8 Trainium2 NeuronCores.

Design (v2):
  - Attention: data-parallel tokens (core c owns tokens [128c,128c+128)).
    k/v are computed REPLICATED on every core (fp8 DoubleRow matmuls from a
    packed x^T) so no kv AllGather is needed. q/scores/ctx/wo for own chunk.
    All 16-bit attention data is FP16 (not bf16) to keep x2 noise low enough
    that router top-2 decisions never flip vs the fp32 reference.
  - rsq (rmsnorm row scales of x) via a tiny early 8-core AllGather.
  - MoE: 4 token-pair groups x 2 expert-halves. Core c = (pair c//2,
    half c%2) computes its 4 experts over the pair's 256 tokens.
    Dispatch: pair AllGather of h2^T as fp8 MAIN + fp8 RESIDUAL planes
    (~fp16 fidelity at fp8 DoubleRow speed; K-chain runs both) + fp32
    router weights (bitcast into the fp8 payload).
    Shared expert: half the IS columns per core, over the pair tokens, in
    FP16, folded into the same down-proj psum accumulation.
    Combine: pair ReduceScatter (fp16) of routed+shared partials.
  - Big matmuls fp8 DoubleRow (157 TMAC/s); fp16 elsewhere; fp32 psum,
    residual, router.
"""

import numpy as np

import concourse.bass as bass
import concourse.bacc as bacc
import concourse.mybir as mybir
import concourse.tile as tile
from concourse.bass_utils import run_bass_kernel_spmd
from concourse.masks import make_identity

F32 = mybir.dt.float32
F16 = mybir.dt.float16
F8 = mybir.dt.float8e4
AF = mybir.ActivationFunctionType
ALU = mybir.AluOpType
AX = mybir.AxisListType
PM = mybir.MatmulPerfMode

N_CORES = 8
T = 1024
TC = 128          # own tokens
TP = 256          # pair tokens
H = 2048
NH = 16
NKV = 4
DH = 128
E = 8
EH = 4            # experts per core
I = 1024
IS = 1024
ISH = IS // 2     # shared cols per core
KH = 16           # 128-tiles over H
KJ = 8            # 256-pair-tiles over H
EPS = 1e-6
SCALE = DH ** -0.5
NEG = -30000.0
EBIAS = -3.0

AS_H = 16.0
WS_GU = 1024.0
SILU_SC = 1.0 / (AS_H * WS_GU)
AS_A = 8.0
WS_D = 1024.0
DQ_D = 1.0 / (AS_A * WS_D)
U_SC = AS_A / (AS_H * WS_GU)
SHD_SC = AS_A * WS_D

RG_ALL = [list(range(N_CORES))]
RG_PAIR = [[0, 1], [2, 3], [4, 5], [6, 7]]

# ag payload (fp8 elements): [h8: KH*128*TC][r8: KH*128*TC][w: TC*E*4 bytes]
H8SZ = KH * 128 * TC
WOFF = 2 * H8SZ
AGSZ = WOFF + TC * E * 4

_cache = {}


def _bc(ap, n, axis=1):
    a = [list(p) for p in ap.ap]
    a.insert(axis, [0, n])
    return bass.AP(tensor=ap.tensor, offset=ap.offset, ap=a)


def build_nc():
    nc = bacc.Bacc("TRN2", target_bir_lowering=False, num_devices=N_CORES)

    x_res = nc.dram_tensor("x_res", [TC, H], F32, kind="ExternalInput")
    xT_in = nc.dram_tensor("xT_in", [KH, 128, T], F16, kind="ExternalInput")
    xTq_in = nc.dram_tensor("xTq_in", [KH, 128, TC], F16,
                            kind="ExternalInput")
    wkv_in = nc.dram_tensor("wkv_in", [KH, 128, 2 * NKV * DH], F16,
                            kind="ExternalInput")
    wq_in = nc.dram_tensor("wq_in", [4, KH, 128, 512], F16,
                           kind="ExternalInput")
    wo_in = nc.dram_tensor("wo_in", [4, KH, 128, 512], F16,
                           kind="ExternalInput")
    rope_k = nc.dram_tensor("rope_k", [8, TC, 4, 64], F16,
                            kind="ExternalInput")
    rope_q = nc.dram_tensor("rope_q", [TC, 4, 64], F16, kind="ExternalInput")
    mask_in = nc.dram_tensor("mask_in", [128, 8, TC], F16,
                             kind="ExternalInput")
    wrT = nc.dram_tensor("wrT", [128, KH, E], F32, kind="ExternalInput")
    wsgu = nc.dram_tensor("wsgu", [KH, 128, 2 * ISH], F16,
                          kind="ExternalInput")
    wsd = nc.dram_tensor("wsd", [ISH // 128, 128, H], F16,
                         kind="ExternalInput")
    wgu_in = nc.dram_tensor("wgu_in", [EH, 16, 128, KJ, 2, 128], F8,
                            kind="ExternalInput")
    wd_in = nc.dram_tensor("wd_in", [EH, 4, 128, 2, H], F8,
                           kind="ExternalInput")
    esel = nc.dram_tensor("esel", [1, EH * E], F32, kind="ExternalInput")
    out_chunk = nc.dram_tensor("out_chunk", [TC, H], F32,
                               kind="ExternalOutput")
    dbg_x2 = nc.dram_tensor("dbg_x2", [TC, H], F32, kind="ExternalOutput")
    dbg_wm = nc.dram_tensor("dbg_wm", [TC, E], F32, kind="ExternalOutput")
    dbg_moe = nc.dram_tensor("dbg_moe", [TC, H], F16, kind="ExternalOutput")

    with tile.TileContext(nc) as tc:
        with tc.tile_pool(name="dram", bufs=1, space="DRAM") as dram, \
             tc.tile_pool(name="const", bufs=1) as const, \
             tc.tile_pool(name="mid", bufs=1) as mid, \
             tc.tile_pool(name="psA", bufs=2, space="PSUM") as psA, \
             tc.tile_pool(name="psB", bufs=2, space="PSUM") as psB, \
             tc.tile_pool(name="pstr", bufs=2, space="PSUM") as pstr:

            # ---- DRAM collective buffers ----
            rsq_in = dram.tile([TC], F32)
            rsq_out = dram.tile([T], F32, addr_space="Shared")
            ag_in = dram.tile([AGSZ], F8)
            ag_out = dram.tile([2 * AGSZ], F8)
            rs_in = dram.tile([TP, H], F16)
            rs_out = dram.tile([TC, H], F16)
            wrow_dram = dram.tile([EH, TP], F32)

            # ---- constants ----
            ident16 = const.tile([128, 128], F16)
            make_identity(nc, ident16)
            ident32 = const.tile([128, 128], F32)
            make_identity(nc, ident32)
            eps_sb = const.tile([128, 1], F32)
            nc.vector.memset(eps_sb, EPS)
            neg3 = const.tile([128, 1], F32)
            nc.vector.memset(neg3, EBIAS)
            esel_sb = const.tile([128, EH * E], F32)
            nc.gpsimd.dma_start(
                out=esel_sb,
                in_=bass.AP(tensor=esel, offset=0, ap=[[0, 128], [1, EH * E]]))
            mask_sb = const.tile([128, 8, TC], F16)
            nc.gpsimd.dma_start(out=mask_sb, in_=mask_in[:, :, :])
            wrT_sb = const.tile([128, KH, E], F32)
            nc.gpsimd.dma_start(out=wrT_sb, in_=wrT[:, :, :])

            # ---- persistent tiles (whole kernel) ----
            x_sb = mid.tile([TC, H], F32)
            x2_sb = mid.tile([TC, H], F32)
            rsqv = mid.tile([128, 8], F32)

            # ================= ATTENTION (scoped pool) =================
            with tc.tile_pool(name="att", bufs=2) as att:
                # --- early loads ---
                nc.sync.dma_start(out=x_sb, in_=x_res[:, :])
                xT_sb = att.tile([128, KH, T], F16, tag="xT", bufs=1)
                wkv_sb = att.tile([128, KH, 1024], F16, tag="wkv", bufs=1)
                for j in range(KH):
                    nc.sync.dma_start(
                        out=xT_sb[:, j, :],
                        in_=bass.AP(tensor=xT_in, offset=j * 128 * T,
                                    ap=[[T, 128], [1, T]]))
                    nc.scalar.dma_start(
                        out=wkv_sb[:, j, :],
                        in_=bass.AP(tensor=wkv_in, offset=j * 128 * 1024,
                                    ap=[[1024, 128], [1, 1024]]))
                xTq_sb = att.tile([128, KH, TC], F16, tag="xTq", bufs=1)
                nc.gpsimd.dma_start(
                    out=xTq_sb,
                    in_=bass.AP(tensor=xTq_in, offset=0,
                                ap=[[TC, 128], [128 * TC, KH], [1, TC]]))
                rope_k_sb = att.tile([128, 8, 4, 64], F16, tag="ropek",
                                     bufs=1)
                nc.gpsimd.dma_start(
                    out=rope_k_sb,
                    in_=bass.AP(tensor=rope_k, offset=0,
                                ap=[[256, 128], [TC * 256, 8], [64, 4],
                                    [1, 64]]))
                rope_q_sb = att.tile([TC, 4, 64], F16, tag="ropeq", bufs=1)
                nc.gpsimd.dma_start(out=rope_q_sb, in_=rope_q[:, :, :])

                # rsq of own chunk -> 8-core AllGather (collective idle now)
                red4 = att.tile([TC, 4], F32, tag="red4", bufs=1)
                for n in range(4):
                    sl = slice(n * 512, (n + 1) * 512)
                    sqp = att.tile([TC, 512], F32, tag="sqp")
                    nc.vector.tensor_mul(sqp, x_sb[:, sl], x_sb[:, sl])
                    nc.vector.tensor_reduce(red4[:, n:n + 1], sqp, axis=AX.X,
                                            op=ALU.add)
                rsqo = att.tile([TC, 1], F32, tag="rsqo", bufs=1)
                nc.vector.tensor_reduce(rsqo, red4, axis=AX.X, op=ALU.add)
                nc.scalar.activation(rsqo, rsqo, AF.Sqrt, bias=eps_sb[:TC],
                                     scale=1.0 / H)
                nc.vector.reciprocal(rsqo, rsqo)
                nc.gpsimd.dma_start(
                    out=bass.AP(tensor=rsq_in.tensor, offset=rsq_in.offset,
                                ap=[[1, TC]]),
                    in_=rsqo.rearrange("p one -> p (one)"))
                nc.gpsimd.collective_compute(
                    "AllGather", ALU.bypass, replica_groups=RG_ALL,
                    ins=[rsq_in.opt()], outs=[rsq_out.opt()])

                # rsq arrives: per-chunk v scale
                nc.gpsimd.dma_start(
                    out=rsqv,
                    in_=bass.AP(tensor=rsq_out.tensor, offset=rsq_out.offset,
                                ap=[[1, 128], [128, 8]]))
    

                kT_all = att.tile([128, NKV, 8, TC], F16, tag="kT", bufs=1)
                v_all = att.tile([128, 8, NKV, DH + 1], F16, tag="v", bufs=1)
                qT_all = att.tile([128, NH, TC], F16, tag="qT", bufs=1)
                ctxT_all = att.tile([128, NH, TC], F16, tag="ctxT", bufs=1)
                h2_f16 = att.tile([TC, H], F16, tag="h2f16", bufs=1)
                h8_sb = att.tile([128, KH, TC], F8, tag="h8", bufs=1)
                r8_sb = att.tile([128, KH, TC], F8, tag="r8", bufs=1)

                def rope(x3, o3, nh, tab):
                    c1 = _bc(tab[:, 0, :], nh)
                    s1 = _bc(tab[:, 1, :], nh)
                    c2 = _bc(tab[:, 2, :], nh)
                    s2 = _bc(tab[:, 3, :], nh)
                    x1 = x3[:, :, 0:64]
                    x2_ = x3[:, :, 64:128]
                    t1 = att.tile([TC, NKV, 64], F32, tag="rp1")
                    tn = att.tile([TC, NKV, 64], F32, tag="rpn")
                    t1v = t1[:, :nh, :]
                    tnv = tn[:, :nh, :]
                    nc.vector.tensor_mul(t1v, x1, c1)
                    nc.gpsimd.tensor_mul(tnv, x2_, s1)
                    nc.vector.tensor_sub(o3[:, :, 0:64], t1v, tnv)
                    nc.gpsimd.tensor_mul(t1v, x2_, c2)
                    nc.vector.tensor_mul(tnv, x1, s2)
                    nc.gpsimd.tensor_add(o3[:, :, 64:128], t1v, tnv)

                def qk_norm(raw3, nh, tag):
                    # raw3 [TC, nh<=4, DH] f32, normed in place
                    sq = att.tile([TC, NKV, DH], F32, tag="nsq")
                    sqv = sq[:, :nh, :]
                    nc.vector.tensor_mul(sqv, raw3, raw3)
                    red = att.tile([TC, NKV, 1], F32, tag=f"nred{tag}")
                    nc.vector.tensor_reduce(red[:, :nh, :], sqv, axis=AX.X,
                                            op=ALU.add)
                    redf = red[:, :nh, :].rearrange("p h one -> p (h one)")
                    nc.scalar.activation(redf, redf, AF.Sqrt,
                                         bias=eps_sb[:TC], scale=1.0 / DH)
                    nc.vector.reciprocal(redf, redf)
                    for h in range(nh):
                        nc.vector.tensor_scalar_mul(
                            raw3[:, h, :], raw3[:, h, :], red[:, h, :])

                # --- k/v for all 8 chunks (replicated); kT transposes
                # run one chunk behind so PE never waits on the DVE chain ---
                def kv_transposes(c, kf):
                    for g in range(NKV):
                        pt = pstr.tile([128, 128], F16, tag="pt")
                        nc.tensor.transpose(pt, kf[:, g, :], ident16)
                        if g % 2 == 0:
                            nc.vector.tensor_copy(kT_all[:, g, c, :], pt)
                        else:
                            nc.scalar.activation(kT_all[:, g, c, :], pt,
                                                 AF.Copy)

                pend_kv = None
                for c in range(8):
                    pkv = psA.tile([128, 1024], F32, tag="p1k")
                    for hf in range(2):
                        for j in range(KH):
                            nc.tensor.matmul(
                                pkv[:, hf * 512:(hf + 1) * 512],
                                xT_sb[:, j, c * TC:(c + 1) * TC],
                                wkv_sb[:, j, hf * 512:(hf + 1) * 512],
                                start=(j == 0), stop=(j == KH - 1))
                    if pend_kv is not None:
                        kv_transposes(*pend_kv)
                    kraw = att.tile([TC, 512], F32, tag="kraw")
                    nc.scalar.activation(kraw, pkv[:, 0:512], AF.Copy,
                                         scale=1.0)
                    k3 = kraw.rearrange("p (h d) -> p h d", h=NKV)
                    qk_norm(k3, NKV, "k")
                    kf = att.tile([TC, NKV, DH], F16, tag="kf")
                    rope(k3, kf, NKV, rope_k_sb[:, c, :, :])
                    nc.vector.tensor_scalar_mul(
                        v_all[:, c, :, 0:DH],
                        pkv[:, 512:1024].rearrange("p (g d) -> p g d", g=NKV),
                        rsqv[:, c:c + 1])
                    nc.gpsimd.memset(v_all[:, c, :, DH:DH + 1], 1.0)
                    pend_kv = (c, kf)
                kv_transposes(*pend_kv)

                # --- q for own chunk, 4 head-groups of 4 ---
                def q_transposes(ng, qf):
                    for hh in range(4):
                        pt = pstr.tile([128, 128], F16, tag="pt")
                        nc.tensor.transpose(pt, qf[:, hh, :], ident16)
                        nc.vector.tensor_copy(qT_all[:, ng * 4 + hh, :], pt)

                pend_q = None
                for ng in range(4):
                    wqs = att.tile([128, KH, 512], F16, tag="wst")
                    nc.scalar.dma_start(
                        out=wqs,
                        in_=bass.AP(tensor=wq_in,
                                    offset=ng * KH * 128 * 512,
                                    ap=[[512, 128], [128 * 512, KH],
                                        [1, 512]]))
                    pq = psB.tile([TC, 512], F32, tag="p512")
                    for j in range(KH):
                        nc.tensor.matmul(
                            pq, xTq_sb[:, j, :], wqs[:, j, :],
                            start=(j == 0), stop=(j == KH - 1))
                    qraw = att.tile([TC, 512], F32, tag="qraw")
                    nc.scalar.activation(qraw, pq, AF.Copy, scale=1.0)
                    q3 = qraw.rearrange("p (h d) -> p h d", h=4)
                    qk_norm(q3, 4, "q")
                    qf = att.tile([TC, 4, DH], F16, tag="qf")
                    rope(q3, qf, 4, rope_q_sb)
                    if pend_q is not None:
                        q_transposes(*pend_q)
                    pend_q = (ng, qf)
                q_transposes(*pend_q)

                # --- per-head scores / ctx (1-deep software pipeline) ---
                def head_front(h):
                    g = h // (NH // NKV)
                    ps = psA.tile([128, 1024], F32, tag="p1k")
                    for c8 in range(8):
                        nc.tensor.matmul(
                            ps[:, c8 * TC:(c8 + 1) * TC],
                            kT_all[:, g, c8, :], qT_all[:, h, :],
                            start=True, stop=True)
                    mf = mask_sb.rearrange("p c q -> p (c q)")
                    nc.vector.tensor_add(ps, ps, mf)
                    pr = att.tile([128, 8, TC], F16, tag="probs")
                    nc.scalar.activation(
                        pr.rearrange("p c q -> p (c q)"), ps, AF.Exp,
                        bias=neg3, scale=SCALE)
                    return pr

                def tail_mm(h, pr):
                    g = h // (NH // NKV)
                    pct = psB.tile([TC, 512], F32, tag="p512")
                    pc = pct[:, 0:DH + 1]
                    for c8 in range(8):
                        nc.tensor.matmul(pc, pr[:, c8, :], v_all[:, c8, g, :],
                                         start=(c8 == 0), stop=(c8 == 7))
                    return pc

                def tail_fin(h, pc):
                    rden = att.tile([TC, 1], F32, tag="rden")
                    nc.vector.reciprocal(rden, pc[:, DH:DH + 1])
                    cf = att.tile([TC, DH], F16, tag="cf")
                    nc.vector.tensor_scalar_mul(cf, pc[:, 0:DH], rden)
                    pt = pstr.tile([128, 128], F16, tag="pt")
                    nc.tensor.transpose(pt, cf, ident16)
                    nc.scalar.activation(ctxT_all[:, h, :], pt,
                                         AF.Copy, scale=1.0)

                pend_mm = None
                pend_fin = None
                for h in range(NH):
                    pr = head_front(h)
                    if pend_mm is not None:
                        pc = tail_mm(*pend_mm)
                        if pend_fin is not None:
                            tail_fin(*pend_fin)
                        pend_fin = (pend_mm[0], pc)
                    pend_mm = (h, pr)
                pc = tail_mm(*pend_mm)
                tail_fin(*pend_fin)
                tail_fin(pend_mm[0], pc)

                # --- wo + residual + rms stats ---
                for n in range(4):
                    wos = att.tile([128, KH, 512], F16, tag="wst")
                    nc.sync.dma_start(
                        out=wos,
                        in_=bass.AP(tensor=wo_in,
                                    offset=n * KH * 128 * 512,
                                    ap=[[512, 128], [128 * 512, KH],
                                        [1, 512]]))
                    po = psB.tile([TC, 512], F32, tag="p512")
                    for j in range(KH):
                        nc.tensor.matmul(
                            po, ctxT_all[:, j, :], wos[:, j, :],
                            start=(j == 0), stop=(j == KH - 1))
                    at16 = att.tile([TC, 512], F16, tag="at16")
                    nc.scalar.activation(at16, po, AF.Copy, scale=1.0)
                    sl = slice(n * 512, (n + 1) * 512)
                    nc.vector.tensor_add(x2_sb[:, sl], at16, x_sb[:, sl])
                    sqp = att.tile([TC, 512], F32, tag="sqp")
                    nc.gpsimd.tensor_mul(sqp, x2_sb[:, sl], x2_sb[:, sl])
                    nc.vector.tensor_reduce(red4[:, n:n + 1], sqp, axis=AX.X,
                                            op=ALU.add)

                # rs2 true + rs2*AS_H
                rs2t = att.tile([TC, 1], F32, tag="rs2t", bufs=1)
                nc.vector.tensor_reduce(rs2t, red4, axis=AX.X, op=ALU.add)
                nc.scalar.activation(rs2t, rs2t, AF.Sqrt, bias=eps_sb[:TC],
                                     scale=1.0 / H)
                nc.vector.reciprocal(rs2t, rs2t)
                rs2h = att.tile([TC, 1], F32, tag="rs2h", bufs=1)
                nc.scalar.activation(rs2h, rs2t, AF.Copy, scale=AS_H)
                nc.vector.tensor_scalar_mul(h2_f16, x2_sb, rs2h)

                # --- router (fp32) interleaved with h8/r8 planes:
                # per j: PE does tr32(j), tr16(j), router-mm(j-1); Act/DVE
                # drain the psums. Keeps every engine fed, AG starts ASAP.
                prlt = psA.tile([128, 1024], F32, tag="p1k")
                prl = prlt[:, 0:E]
                pend_r = None
                for j in range(KH):
                    ptft = psB.tile([TC, 512], F32, tag="p512")
                    ptf = ptft[:, 0:128]
                    nc.tensor.transpose(
                        ptf, x2_sb[:, j * 128:(j + 1) * 128], ident32)
                    pt = pstr.tile([128, 128], F16, tag="pt")
                    nc.tensor.transpose(
                        pt, h2_f16[:, j * 128:(j + 1) * 128], ident16)
                    if pend_r is not None:
                        nc.tensor.matmul(prl, pend_r, wrT_sb[:, j - 1, :],
                                         start=(j == 1), stop=False)
                    x2T = att.tile([128, TC], F32, tag="x2T")
                    nc.vector.tensor_copy(x2T, ptf)
                    pend_r = x2T
                    nc.scalar.activation(h8_sb[:, j, :], pt, AF.Copy,
                                         scale=1.0)
                    rr16 = att.tile([128, TC], F16, tag="rr16")
                    nc.vector.tensor_sub(rr16, pt, h8_sb[:, j, :])
                    nc.scalar.activation(r8_sb[:, j, :], rr16, AF.Copy,
                                         scale=1.0)
                nc.tensor.matmul(prl, pend_r, wrT_sb[:, KH - 1, :],
                                 start=False, stop=True)
                nc.sync.dma_start(
                    out=bass.AP(tensor=ag_in.tensor, offset=ag_in.offset,
                                ap=[[TC, 128], [128 * TC, KH], [1, TC]]),
                    in_=h8_sb)
                nc.scalar.dma_start(
                    out=bass.AP(tensor=ag_in.tensor,
                                offset=ag_in.offset + H8SZ,
                                ap=[[TC, 128], [128 * TC, KH], [1, TC]]),
                    in_=r8_sb)
                lg = att.tile([TC, E], F32, tag="lg", bufs=1)
                nc.vector.tensor_scalar_mul(lg, prl, rs2t)
                probs8 = att.tile([TC, E], F32, tag="probs8", bufs=1)
                nc.scalar.activation(probs8, lg, AF.Exp, scale=1.0)
                den8 = att.tile([TC, 1], F32, tag="den8", bufs=1)
                nc.vector.tensor_reduce(den8, probs8, axis=AX.X, op=ALU.add)
                nc.vector.reciprocal(den8, den8)
                nc.vector.tensor_scalar_mul(probs8, probs8, den8)
                mx8 = att.tile([TC, 8], F32, tag="mx8", bufs=1)
                nc.vector.max(out=mx8, in_=probs8)
                s12 = att.tile([TC, 1], F32, tag="s12", bufs=1)
                nc.vector.tensor_add(s12, mx8[:, 0:1], mx8[:, 1:2])
                nc.vector.reciprocal(s12, s12)
                eq1 = att.tile([TC, E], F32, tag="eq1", bufs=1)
                nc.vector.tensor_scalar(eq1, probs8, mx8[:, 0:1], None,
                                        op0=ALU.is_equal)
                eq2 = att.tile([TC, E], F32, tag="eq2", bufs=1)
                nc.vector.tensor_scalar(eq2, probs8, mx8[:, 1:2], None,
                                        op0=ALU.is_equal)
                nc.vector.tensor_add(eq1, eq1, eq2)
                wm = att.tile([TC, E], F32, tag="wm", bufs=1)
                nc.vector.tensor_mul(wm, probs8, eq1)
                nc.vector.tensor_scalar_mul(wm, wm, s12)
                nc.gpsimd.dma_start(
                    out=bass.AP(tensor=ag_in.tensor,
                                offset=ag_in.offset + WOFF,
                                ap=[[E * 4, TC], [1, E * 4]]),
                    in_=wm.bitcast(F8))
                nc.gpsimd.dma_start(out=dbg_wm[:, :], in_=wm)
                nc.gpsimd.dma_start(out=dbg_x2[:, :], in_=x2_sb)


            nc.gpsimd.collective_compute(
                "AllGather", ALU.bypass, replica_groups=RG_PAIR,
                ins=[ag_in.opt()], outs=[ag_out.opt()])

            # ================= MOE =================
            with tc.tile_pool(name="moe", bufs=2) as moe, \
                 tc.tile_pool(name="wgup", bufs=3) as wgup:

                # weights loaded in the AG window (DMA is idle there)
                wsgu_sb = moe.tile([128, KH, 2 * ISH], F16, tag="wsgu",
                                   bufs=1)
                nc.scalar.dma_start(
                    out=wsgu_sb,
                    in_=bass.AP(tensor=wsgu, offset=0,
                                ap=[[2 * ISH, 128], [128 * 2 * ISH, KH],
                                    [1, 2 * ISH]]))
                wsd_sb = moe.tile([128, ISH // 128, H], F16, tag="wsd",
                                  bufs=1)
                nc.sync.dma_start(
                    out=wsd_sb,
                    in_=bass.AP(tensor=wsd, offset=0,
                                ap=[[H, 128], [128 * H, ISH // 128],
                                    [1, H]]))
                wd_sb = moe.tile([128, EH * 4, 2, H], F8, tag="wd", bufs=1)
                for s in range(EH):
                    (nc.sync if s % 2 == 0 else nc.scalar).dma_start(
                        out=wd_sb[:, s * 4:(s + 1) * 4, :, :],
                        in_=bass.AP(tensor=wd_in, offset=s * 4 * 128 * 2 * H,
                                    ap=[[2 * H, 128], [128 * 2 * H, 4],
                                        [H, 2], [1, H]]))

                # gathered h8/r8 pair planes [128, jj, pl, tok(rank-major)]
                h8P = moe.tile([128, KJ, 2, TP], F8, tag="h8P", bufs=1)
                r8P = moe.tile([128, KJ, 2, TP], F8, tag="r8P", bufs=1)
                for r in range(2):
                    base = ag_out.offset + r * AGSZ
                    nc.gpsimd.dma_start(
                        out=h8P[:, :, :, r * TC:(r + 1) * TC],
                        in_=bass.AP(tensor=ag_out.tensor, offset=base,
                                    ap=[[TC, 128], [2 * 128 * TC, KJ],
                                        [128 * TC, 2], [1, TC]]))
                    nc.gpsimd.dma_start(
                        out=r8P[:, :, :, r * TC:(r + 1) * TC],
                        in_=bass.AP(tensor=ag_out.tensor, offset=base + H8SZ,
                                    ap=[[TC, 128], [2 * 128 * TC, KJ],
                                        [128 * TC, 2], [1, TC]]))
                # router weights of the pair
                wpair = moe.tile([128, 2, E], F32, tag="wpair", bufs=1)
                for r in range(2):
                    nc.gpsimd.dma_start(
                        out=wpair[:, r, :],
                        in_=bass.AP(tensor=ag_out.tensor,
                                    offset=ag_out.offset + r * AGSZ + WOFF,
                                    ap=[[E * 4, TC], [1, E * 4]]).bitcast(F32))

                # reconstruct fp16 h2 pair (for shared expert)
                h2P16 = moe.tile([128, KJ, 2, TP], F16, tag="h2P16", bufs=1)
                nc.vector.tensor_add(
                    h2P16.rearrange("p a b t -> p (a b t)"),
                    h8P.rearrange("p a b t -> p (a b t)"),
                    r8P.rearrange("p a b t -> p (a b t)"))

                # per-expert token weight rows -> broadcast tiles
                wcols = moe.tile([128, 2, EH], F32, tag="wcols", bufs=1)
                for s in range(EH):
                    wtmp = moe.tile([128, 2, E], F32, tag="wtmp")
                    nc.vector.tensor_mul(
                        wtmp, wpair,
                        _bc(esel_sb[:, s * E:(s + 1) * E], 2))
                    nc.vector.tensor_reduce(wcols[:, :, s:s + 1], wtmp,
                                            axis=AX.X, op=ALU.add)
                for r in range(2):
                    nc.gpsimd.dma_start(
                        out=bass.AP(tensor=wrow_dram.tensor,
                                    offset=wrow_dram.offset + r * TC,
                                    ap=[[1, 128], [TP, EH]]),
                        in_=wcols[:, r, :])
                w_bcast = moe.tile([128, EH, TP], F32, tag="w_bcast", bufs=1)
                for s in range(EH):
                    nc.gpsimd.dma_start(
                        out=w_bcast[:, s, :],
                        in_=bass.AP(tensor=wrow_dram.tensor,
                                    offset=wrow_dram.offset + s * TP,
                                    ap=[[0, 128], [1, TP]]))

                # --- shared expert (fp16, half IS, pair tokens) ---
                actsT_sh = moe.tile([128, ISH // 128, TP], F16, tag="actsTsh",
                                    bufs=1)
                for ch in range(2):
                    psg = psA.tile([128, 1024], F32, tag="p1k")
                    for hf in range(2):
                        for j16 in range(KH):
                            nc.tensor.matmul(
                                psg[:, hf * 512:(hf + 1) * 512],
                                h2P16[:, j16 // 2, j16 % 2,
                                      ch * TC:(ch + 1) * TC],
                                wsgu_sb[:, j16, hf * 512:(hf + 1) * 512],
                                start=(j16 == 0), stop=(j16 == KH - 1))
                    gsh = moe.tile([128, ISH], F16, tag="gsh")
                    nc.scalar.activation(gsh, psg[:, 0:ISH], AF.Silu,
                                         scale=1.0 / AS_H)
                    ush = moe.tile([128, ISH], F16, tag="ush")
                    nc.scalar.activation(ush, psg[:, ISH:2 * ISH], AF.Copy,
                                         scale=1.0 / AS_H)
                    acts = moe.tile([128, ISH], F16, tag="acts")
                    nc.vector.tensor_mul(acts, gsh, ush)
                    for it in range(ISH // 128):
                        pt = pstr.tile([128, 128], F16, tag="pt")
                        nc.tensor.transpose(
                            pt, acts[:, it * 128:(it + 1) * 128], ident16)
                        if it % 2 == 0:
                            nc.vector.tensor_copy(
                                actsT_sh[:, it, ch * TC:(ch + 1) * TC], pt)
                        else:
                            nc.scalar.activation(
                                actsT_sh[:, it, ch * TC:(ch + 1) * TC], pt,
                                AF.Copy)

                # --- routed experts: gu (fp8 DR, h8+r8 K-chain) ---
                act_pr = moe.tile([128, EH * 4, 2, TP], F8, tag="act_pr",
                                  bufs=1)
                for s in range(EH):
                    g16 = moe.tile([128, 8, TP], F16, tag="g16")
                    wkc = None
                    for mi in range(16):
                        if mi % 2 == 0:
                            wkc = wgup.tile([128, 2, KJ, 2, 128], F8,
                                            tag="wgu")
                            (nc.sync if mi % 4 == 0 else nc.scalar).dma_start(
                                out=wkc,
                                in_=bass.AP(
                                    tensor=wgu_in,
                                    offset=(s * 16 + mi) * 128 * 2048,
                                    ap=[[2048, 128], [128 * 2048, 2],
                                        [256, KJ], [128, 2], [1, 128]]))
                        pgt = psB.tile([TC, 512], F32, tag="p512")
                        pg = pgt[:, 0:TP]
                        for jj in range(KJ):
                            nc.tensor.matmul(
                                pg, wkc[:, mi % 2, jj, :, :],
                                h8P[:, jj, :, :],
                                start=(jj == 0), stop=False,
                                perf_mode=PM.DoubleRow)
                        for jj in range(KJ):
                            nc.tensor.matmul(
                                pg, wkc[:, mi % 2, jj, :, :],
                                r8P[:, jj, :, :],
                                start=False, stop=(jj == KJ - 1),
                                perf_mode=PM.DoubleRow)
                        if mi < 8:
                            nc.scalar.activation(g16[:, mi, :], pg, AF.Silu,
                                                 scale=SILU_SC)
                        else:
                            iu = mi - 8
                            u16 = moe.tile([128, TP], F16, tag="u16", bufs=3)
                            nc.vector.tensor_mul(u16, pg, w_bcast[:, s, :])
                            (nc.vector if iu % 2 == 0
                             else nc.gpsimd).tensor_mul(
                                act_pr[:, s * 4 + iu // 2, iu % 2, :],
                                g16[:, iu, :], u16)

                # --- down proj (routed fp8 DR + shared fp16 in one psum) ---
                for ch in range(2):
                    for n in range(4):
                        pd = psB.tile([TC, 512], F32, tag="p512")
                        for kk in range(EH * 4):
                            nc.tensor.matmul(
                                pd,
                                act_pr[:, kk, :, ch * TC:(ch + 1) * TC],
                                wd_sb[:, kk, :, n * 512:(n + 1) * 512],
                                start=(kk == 0), stop=False,
                                perf_mode=PM.DoubleRow)
                        for it in range(ISH // 128):
                            nc.tensor.matmul(
                                pd,
                                actsT_sh[:, it, ch * TC:(ch + 1) * TC],
                                wsd_sb[:, it, n * 512:(n + 1) * 512],
                                start=False, stop=(it == ISH // 128 - 1))
                        rsd = moe.tile([TC, 512], F16, tag="rsd", bufs=3)
                        nc.scalar.activation(rsd, pd, AF.Copy, scale=DQ_D)
                        (nc.sync if (ch * 4 + n) % 2 == 0
                         else nc.scalar).dma_start(
                            out=rs_in[ch * TC:(ch + 1) * TC,
                                      n * 512:(n + 1) * 512],
                            in_=rsd)

                nc.gpsimd.collective_compute(
                    "ReduceScatter", ALU.add, replica_groups=RG_PAIR,
                    ins=[rs_in.opt()], outs=[rs_out.opt()])

                # --- combine + residual ---
                for q in range(4):
                    sl = slice(q * 512, (q + 1) * 512)
                    rsld = moe.tile([TC, 512], F16, tag="rsld")
                    (nc.sync if q % 2 == 0 else nc.scalar).dma_start(
                        out=rsld, in_=rs_out[:, sl])
                    outf = moe.tile([TC, 512], F32, tag="outf")
                    nc.vector.tensor_add(outf, x2_sb[:, sl], rsld)
                    (nc.sync if q % 2 == 0 else nc.scalar).dma_start(
                        out=out_chunk[:, sl], in_=outf)
                    nc.gpsimd.dma_start(out=dbg_moe[:, sl], in_=rsld)

    nc.compile()
    return nc


def _prep_inputs(hidden_states, w_ln1, w_ln2, wqkv, q_norm_w, k_norm_w, wo,
                 w_router, w_gu, w_d, ws_gu, ws_d, positions):
    import ml_dtypes
    f16 = np.float16
    f8 = ml_dtypes.float8_e4m3

    x = np.asarray(hidden_states, np.float32).reshape(T, H)
    w_ln1 = np.asarray(w_ln1, np.float32)
    w_ln2 = np.asarray(w_ln2, np.float32)
    wqkv_e = np.asarray(wqkv, np.float32) * w_ln1[:, None]  # [H, 3072]

    def pack_dr(W, scale):
        # W [Hrows, C] -> [KJ, 128, 2, C]; row = 256*jj + 128*pl + r
        Wr = (np.asarray(W, np.float32) * scale).reshape(KJ, 2, 128, -1)
        return np.ascontiguousarray(Wr.transpose(0, 2, 1, 3)).astype(f8)

    def pack16(W):
        # W [Hrows, C] -> [KH, 128, C] fp16
        return np.ascontiguousarray(
            np.asarray(W, np.float32).reshape(KH, 128, -1)).astype(f16)

    def by_coltile16(W):
        # W [Hrows, C] -> [C//512, KH, 128, 512] fp16
        C = W.shape[1]
        return np.ascontiguousarray(
            np.asarray(W, np.float32).reshape(KH, 128, C // 512, 512)
            .transpose(2, 0, 1, 3)).astype(f16)

    wq_p = by_coltile16(wqkv_e[:, :NH * DH])
    wkv_p = pack16(wqkv_e[:, NH * DH:])
    xT_p = pack16(x.T)
    wo_p = by_coltile16(np.asarray(wo, np.float32))

    pos = np.asarray(positions).astype(np.float64)
    inv_freq = 1.0 / (10000.0 ** (np.arange(0, DH, 2, dtype=np.float64) / DH))
    freqs = pos[:, None] * inv_freq[None, :]
    cos = np.cos(freqs).astype(np.float32)
    sin = np.sin(freqs).astype(np.float32)
    qw = np.asarray(q_norm_w, np.float32)
    kw = np.asarray(k_norm_w, np.float32)

    def rope_tab(w):
        return np.ascontiguousarray(
            np.stack([cos * w[None, :64], sin * w[None, 64:],
                      cos * w[None, 64:], sin * w[None, :64]],
                     axis=1)).astype(f16)

    rq = rope_tab(qw)  # [T, 4, 64]
    rk = rope_tab(kw).reshape(8, TC, 4, 64)

    wrT_e = (np.asarray(w_router, np.float32) * w_ln2[None, :]).T  # [H, E]
    wrT_p = np.ascontiguousarray(
        wrT_e.reshape(KH, 128, E).transpose(1, 0, 2)).astype(np.float32)

    ws_gu_e = np.asarray(ws_gu, np.float32) * w_ln2[:, None]
    ws_d_e = np.asarray(ws_d, np.float32) * SHD_SC
    w_gu_e = np.asarray(w_gu, np.float32) * w_ln2[None, :, None] * WS_GU
    w_d_e = np.asarray(w_d, np.float32) * WS_D

    kidx = np.arange(T)
    in_maps = []
    for c in range(N_CORES):
        rows = np.arange(c * TC, (c + 1) * TC)
        # mask[p, kc, q]: k token = kc*128+p, q token = c*128+q
        kk = kidx.reshape(8, 128)
        mask = np.where(kk.T[:, :, None] <= rows[None, None, :], 0.0, NEG)
        mask = np.ascontiguousarray(mask).astype(f16)

        ph = c % 2
        # shared half: own g/u columns
        wsgu_half = np.concatenate(
            [ws_gu_e[:, ph * ISH:(ph + 1) * ISH],
             ws_gu_e[:, IS + ph * ISH:IS + (ph + 1) * ISH]], axis=1)
        wsgu_p = np.ascontiguousarray(
            wsgu_half.reshape(KH, 128, 2 * ISH)).astype(f16)
        wsd_p = np.ascontiguousarray(
            ws_d_e[ph * ISH:(ph + 1) * ISH].reshape(ISH // 128, 128, H)
        ).astype(f16)

        # routed experts for this half
        es = np.zeros((1, EH * E), np.float32)
        wgu_p = np.empty((EH, 16, 128, KJ, 2, 128), f8)
        wd_p = np.empty((EH, 4, 128, 2, H), f8)
        for s in range(EH):
            e = ph * EH + s
            es[0, s * E + e] = U_SC
            wg = w_gu_e[e].reshape(KJ, 2, 128, 16, 128)  # [jj, pl, r, mi, m]
            wgu_p[s] = wg.transpose(3, 2, 0, 1, 4).astype(f8)
            wdv = w_d_e[e].reshape(4, 2, 128, H)  # [j, pl, r, h]
            wd_p[s] = wdv.transpose(0, 2, 1, 3).astype(f8)

        in_maps.append({
            "x_res": np.ascontiguousarray(x[c * TC:(c + 1) * TC]),
            "xT_in": xT_p,
            "xTq_in": np.ascontiguousarray(
                xT_p[:, :, c * TC:(c + 1) * TC]),
            "wkv_in": wkv_p,
            "wq_in": wq_p,
            "wo_in": wo_p,
            "rope_k": rk,
            "rope_q": np.ascontiguousarray(rq[c * TC:(c + 1) * TC]),
            "mask_in": mask,
            "wrT": wrT_p,
            "wsgu": wsgu_p,
            "wsd": wsd_p,
            "wgu_in": np.ascontiguousarray(wgu_p),
            "wd_in": np.ascontiguousarray(wd_p),
            "esel": es,
        })
    return in_maps


def kernel(**inputs):
    import os
    if "nc" not in _cache:
        _cache["nc"] = build_nc()
    nc = _cache["nc"]
    in_maps = _prep_inputs(**inputs)
    trace = bool(int(os.environ.get("KERNEL_TRACE", "0")))
    res = run_bass_kernel_spmd(nc, in_maps, core_ids=list(range(N_CORES)),
                               trace=trace)
    _cache["last_result"] = res
    out = np.concatenate(
        [res.results[c]["out_chunk"] for c in range(N_CORES)], axis=0)
    return out.reshape(1, T, H).astype(np.float32)


if __name__ == "__main__":
    import reference
    inp = {k: np.asarray(v) for k, v in reference.setup_inputs().items()}
    got = kernel(**inp)
    exp = np.asarray(reference.reference(**reference.setup_inputs()))
    denom = np.abs(exp).max()
    err = np.abs(got - exp).max() / denom
    print("abs max:", denom, "rel err:", err)


# revision 23
# speedup vs baseline: 1.0629x; 1.0249x over previous
"""BailingMoE block on 8 Trainium2 NeuronCores.

Design (v2):
  - Attention: data-parallel tokens (core c owns tokens [128c,128c+128)).
    k/v are computed REPLICATED on every core (fp8 DoubleRow matmuls from a
    packed x^T) so no kv AllGather is needed. q/scores/ctx/wo for own chunk.
    All 16-bit attention data is FP16 (not bf16) to keep x2 noise low enough
    that router top-2 decisions never flip vs the fp32 reference.
  - rsq (rmsnorm row scales of x) via a tiny early 8-core AllGather.
  - MoE: 4 token-pair groups x 2 expert-halves. Core c = (pair c//2,
    half c%2) computes its 4 experts over the pair's 256 tokens.
    Dispatch: pair AllGather of h2^T as fp8 MAIN + fp8 RESIDUAL planes
    (~fp16 fidelity at fp8 DoubleRow speed; K-chain runs both) + fp32
    router weights (bitcast into the fp8 payload).
    Shared expert: half the IS columns per core, over the pair tokens, in
    FP16, folded into the same down-proj psum accumulation.
    Combine: pair ReduceScatter (fp16) of routed+shared partials.
  - Big matmuls fp8 DoubleRow (157 TMAC/s); fp16 elsewhere; fp32 psum,
    residual, router.
"""

import numpy as np

import concourse.bass as bass
import concourse.bacc as bacc
import concourse.mybir as mybir
import concourse.tile as tile
from concourse.bass_utils import run_bass_kernel_spmd
from concourse.masks import make_identity

F32 = mybir.dt.float32
F16 = mybir.dt.float16
F8 = mybir.dt.float8e4
AF = mybir.ActivationFunctionType
ALU = mybir.AluOpType
AX = mybir.AxisListType
PM = mybir.MatmulPerfMode

N_CORES = 8
T = 1024
TC = 128          # own tokens
TP = 256          # pair tokens
H = 2048
NH = 16
NKV = 4
DH = 128
E = 8
EH = 4            # experts per core
I = 1024
IS = 1024
ISH = IS // 2     # shared cols per core
KH = 16           # 128-tiles over H
KJ = 8            # 256-pair-tiles over H
EPS = 1e-6
SCALE = DH ** -0.5
NEG = -30000.0
EBIAS = -3.0

AS_H = 16.0
WS_GU = 1024.0
SILU_SC = 1.0 / (AS_H * WS_GU)
AS_A = 8.0
WS_D = 1024.0
DQ_D = 1.0 / (AS_A * WS_D)
U_SC = AS_A / (AS_H * WS_GU)
SHD_SC = AS_A * WS_D

RG_ALL = [list(range(N_CORES))]
RG_PAIR = [[0, 1], [2, 3], [4, 5], [6, 7]]

# ag payload (fp8 elements): [h8: KH*128*TC][r8: KH*128*TC][w: TC*E*4 bytes]
H8SZ = KH * 128 * TC
WOFF = 2 * H8SZ
AGSZ = WOFF + TC * E * 4

_cache = {}


def _bc(ap, n, axis=1):
    a = [list(p) for p in ap.ap]
    a.insert(axis, [0, n])
    return bass.AP(tensor=ap.tensor, offset=ap.offset, ap=a)


def build_nc():
    nc = bacc.Bacc("TRN2", target_bir_lowering=False, num_devices=N_CORES)

    x_res = nc.dram_tensor("x_res", [TC, H], F32, kind="ExternalInput")
    xT_in = nc.dram_tensor("xT_in", [KH, 128, T], F16, kind="ExternalInput")
    xTq_in = nc.dram_tensor("xTq_in", [KH, 128, TC], F16,
                            kind="ExternalInput")
    wkv_in = nc.dram_tensor("wkv_in", [KH, 128, 2 * NKV * DH], F16,
                            kind="ExternalInput")
    wq_in = nc.dram_tensor("wq_in", [4, KH, 128, 512], F16,
                           kind="ExternalInput")
    wo_in = nc.dram_tensor("wo_in", [4, KH, 128, 512], F16,
                           kind="ExternalInput")
    rope_k = nc.dram_tensor("rope_k", [8, TC, 4, 64], F16,
                            kind="ExternalInput")
    rope_q = nc.dram_tensor("rope_q", [TC, 4, 64], F16, kind="ExternalInput")
    mask_in = nc.dram_tensor("mask_in", [128, 8, TC], F16,
                             kind="ExternalInput")
    wrT = nc.dram_tensor("wrT", [128, KH, E], F32, kind="ExternalInput")
    wsgu = nc.dram_tensor("wsgu", [KH, 128, 2 * ISH], F16,
                          kind="ExternalInput")
    wsd = nc.dram_tensor("wsd", [ISH // 128, 128, H], F16,
                         kind="ExternalInput")
    wgu_in = nc.dram_tensor("wgu_in", [EH, 16, 128, KJ, 2, 128], F8,
                            kind="ExternalInput")
    wd_in = nc.dram_tensor("wd_in", [EH, 4, 128, 2, H], F8,
                           kind="ExternalInput")
    esel = nc.dram_tensor("esel", [1, EH * E], F32, kind="ExternalInput")
    out_chunk = nc.dram_tensor("out_chunk", [TC, H], F32,
                               kind="ExternalOutput")
    dbg_x2 = nc.dram_tensor("dbg_x2", [TC, H], F32, kind="ExternalOutput")
    dbg_wm = nc.dram_tensor("dbg_wm", [TC, E], F32, kind="ExternalOutput")
    dbg_moe = nc.dram_tensor("dbg_moe", [TC, H], F16, kind="ExternalOutput")

    with tile.TileContext(nc) as tc:
        with tc.tile_pool(name="dram", bufs=1, space="DRAM") as dram, \
             tc.tile_pool(name="const", bufs=1) as const, \
             tc.tile_pool(name="mid", bufs=1) as mid, \
             tc.tile_pool(name="psA", bufs=2, space="PSUM") as psA, \
             tc.tile_pool(name="psB", bufs=2, space="PSUM") as psB, \
             tc.tile_pool(name="pstr", bufs=2, space="PSUM") as pstr:

            # ---- DRAM collective buffers ----
            rsq_in = dram.tile([TC], F32)
            rsq_out = dram.tile([T], F32, addr_space="Shared")
            ag_in = dram.tile([AGSZ], F8)
            ag_out = dram.tile([2 * AGSZ], F8)
            rs_in = dram.tile([TP, H], F16)
            rs_out = dram.tile([TC, H], F16)
            wrow_dram = dram.tile([EH, TP], F32)

            # ---- constants ----
            ident16 = const.tile([128, 128], F16)
            make_identity(nc, ident16)
            ident32 = const.tile([128, 128], F32)
            make_identity(nc, ident32)
            eps_sb = const.tile([128, 1], F32)
            nc.vector.memset(eps_sb, EPS)
            neg3 = const.tile([128, 1], F32)
            nc.vector.memset(neg3, EBIAS)
            esel_sb = const.tile([128, EH * E], F32)
            nc.gpsimd.dma_start(
                out=esel_sb,
                in_=bass.AP(tensor=esel, offset=0, ap=[[0, 128], [1, EH * E]]))
            mask_sb = const.tile([128, 8, TC], F16)
            nc.sync.dma_start(out=mask_sb, in_=mask_in[:, :, :])
            wrT_sb = const.tile([128, KH, E], F32)
            nc.gpsimd.dma_start(out=wrT_sb, in_=wrT[:, :, :])

            # ---- persistent tiles (whole kernel) ----
            x_sb = mid.tile([TC, H], F32)
            x2_sb = mid.tile([TC, H], F32)
            rsqv = mid.tile([128, 8], F32)

            # ================= ATTENTION (scoped pool) =================
            with tc.tile_pool(name="att", bufs=2) as att:
                # --- early loads ---
                nc.sync.dma_start(out=x_sb, in_=x_res[:, :])
                xT_sb = att.tile([128, KH, T], F16, tag="xT", bufs=1)
                wkv_sb = att.tile([128, KH, 1024], F16, tag="wkv", bufs=1)
                for j in range(KH):
                    nc.sync.dma_start(
                        out=xT_sb[:, j, :],
                        in_=bass.AP(tensor=xT_in, offset=j * 128 * T,
                                    ap=[[T, 128], [1, T]]))
                    nc.scalar.dma_start(
                        out=wkv_sb[:, j, :],
                        in_=bass.AP(tensor=wkv_in, offset=j * 128 * 1024,
                                    ap=[[1024, 128], [1, 1024]]))
                xTq_sb = att.tile([128, KH, TC], F16, tag="xTq", bufs=1)
                nc.scalar.dma_start(
                    out=xTq_sb,
                    in_=bass.AP(tensor=xTq_in, offset=0,
                                ap=[[TC, 128], [128 * TC, KH], [1, TC]]))
                rope_k_sb = att.tile([128, 8, 4, 64], F16, tag="ropek",
                                     bufs=1)
                nc.scalar.dma_start(
                    out=rope_k_sb,
                    in_=bass.AP(tensor=rope_k, offset=0,
                                ap=[[256, 128], [TC * 256, 8], [64, 4],
                                    [1, 64]]))
                rope_q_sb = att.tile([TC, 4, 64], F16, tag="ropeq", bufs=1)
                nc.scalar.dma_start(out=rope_q_sb, in_=rope_q[:, :, :])

                # rsq of own chunk -> 8-core AllGather (collective idle now)
                red4 = att.tile([TC, 4], F32, tag="red4", bufs=1)
                for n in range(4):
                    sl = slice(n * 512, (n + 1) * 512)
                    sqp = att.tile([TC, 512], F32, tag="sqp")
                    nc.vector.tensor_mul(sqp, x_sb[:, sl], x_sb[:, sl])
                    nc.vector.tensor_reduce(red4[:, n:n + 1], sqp, axis=AX.X,
                                            op=ALU.add)
                rsqo = att.tile([TC, 1], F32, tag="rsqo", bufs=1)
                nc.vector.tensor_reduce(rsqo, red4, axis=AX.X, op=ALU.add)
                nc.scalar.activation(rsqo, rsqo, AF.Sqrt, bias=eps_sb[:TC],
                                     scale=1.0 / H)
                nc.vector.reciprocal(rsqo, rsqo)
                nc.gpsimd.dma_start(
                    out=bass.AP(tensor=rsq_in.tensor, offset=rsq_in.offset,
                                ap=[[1, TC]]),
                    in_=rsqo.rearrange("p one -> p (one)"))
                nc.gpsimd.collective_compute(
                    "AllGather", ALU.bypass, replica_groups=RG_ALL,
                    ins=[rsq_in.opt()], outs=[rsq_out.opt()])

                # rsq arrives: per-chunk v scale
                nc.gpsimd.dma_start(
                    out=rsqv,
                    in_=bass.AP(tensor=rsq_out.tensor, offset=rsq_out.offset,
                                ap=[[1, 128], [128, 8]]))
    

                kT_all = att.tile([128, NKV, 8, TC], F16, tag="kT", bufs=1)
                v_all = att.tile([128, 8, NKV, DH + 1], F16, tag="v", bufs=1)
                qT_all = att.tile([128, NH, TC], F16, tag="qT", bufs=1)
                ctxT_all = att.tile([128, NH, TC], F16, tag="ctxT", bufs=1)
                h2_f16 = att.tile([TC, H], F16, tag="h2f16", bufs=1)
                h8_sb = att.tile([128, KH, TC], F8, tag="h8", bufs=1)
                r8_sb = att.tile([128, KH, TC], F8, tag="r8", bufs=1)

                def rope(x3, o3, nh, tab):
                    c1 = _bc(tab[:, 0, :], nh)
                    s1 = _bc(tab[:, 1, :], nh)
                    c2 = _bc(tab[:, 2, :], nh)
                    s2 = _bc(tab[:, 3, :], nh)
                    x1 = x3[:, :, 0:64]
                    x2_ = x3[:, :, 64:128]
                    t1 = att.tile([TC, NKV, 64], F32, tag="rp1")
                    tn = att.tile([TC, NKV, 64], F32, tag="rpn")
                    t1v = t1[:, :nh, :]
                    tnv = tn[:, :nh, :]
                    nc.vector.tensor_mul(t1v, x1, c1)
                    nc.gpsimd.tensor_mul(tnv, x2_, s1)
                    nc.vector.tensor_sub(o3[:, :, 0:64], t1v, tnv)
                    nc.gpsimd.tensor_mul(t1v, x2_, c2)
                    nc.vector.tensor_mul(tnv, x1, s2)
                    nc.gpsimd.tensor_add(o3[:, :, 64:128], t1v, tnv)

                def qk_norm(raw3, nh, tag):
                    # raw3 [TC, nh<=4, DH] f32, normed in place
                    sq = att.tile([TC, NKV, DH], F32, tag="nsq")
                    sqv = sq[:, :nh, :]
                    nc.vector.tensor_mul(sqv, raw3, raw3)
                    red = att.tile([TC, NKV, 1], F32, tag=f"nred{tag}")
                    nc.vector.tensor_reduce(red[:, :nh, :], sqv, axis=AX.X,
                                            op=ALU.add)
                    redf = red[:, :nh, :].rearrange("p h one -> p (h one)")
                    nc.scalar.activation(redf, redf, AF.Sqrt,
                                         bias=eps_sb[:TC], scale=1.0 / DH)
                    nc.vector.reciprocal(redf, redf)
                    for h in range(nh):
                        nc.vector.tensor_scalar_mul(
                            raw3[:, h, :], raw3[:, h, :], red[:, h, :])

                # --- k/v for all 8 chunks (replicated); kT transposes
                # run one chunk behind so PE never waits on the DVE chain ---
                def kv_transposes(c, kf):
                    for g in range(NKV):
                        pt = pstr.tile([128, 128], F16, tag="pt")
                        nc.tensor.transpose(pt, kf[:, g, :], ident16)
                        if g % 2 == 0:
                            nc.vector.tensor_copy(kT_all[:, g, c, :], pt)
                        else:
                            nc.scalar.activation(kT_all[:, g, c, :], pt,
                                                 AF.Copy)

                nc.vector.memset(v_all[:, :, :, DH:DH + 1], 1.0)
                pend_kv = None
                for c in range(8):
                    pkv = psA.tile([128, 1024], F32, tag="p1k")
                    for hf in range(2):
                        for j in range(KH):
                            nc.tensor.matmul(
                                pkv[:, hf * 512:(hf + 1) * 512],
                                xT_sb[:, j, c * TC:(c + 1) * TC],
                                wkv_sb[:, j, hf * 512:(hf + 1) * 512],
                                start=(j == 0), stop=(j == KH - 1))
                    if pend_kv is not None:
                        kv_transposes(*pend_kv)
                    kraw = att.tile([TC, 512], F32, tag="kraw")
                    nc.scalar.activation(kraw, pkv[:, 0:512], AF.Copy,
                                         scale=1.0)
                    k3 = kraw.rearrange("p (h d) -> p h d", h=NKV)
                    qk_norm(k3, NKV, "k")
                    kf = att.tile([TC, NKV, DH], F16, tag="kf")
                    rope(k3, kf, NKV, rope_k_sb[:, c, :, :])
                    nc.scalar.activation(
                        v_all[:, c, :, 0:DH],
                        pkv[:, 512:1024].rearrange("p (g d) -> p g d", g=NKV),
                        AF.Copy, scale=1.0)
                    pend_kv = (c, kf)
                kv_transposes(*pend_kv)
                # deferred v scaling by gathered rsq (decouples kv loop
                # from the rsq AllGather round-trip)
                for c in range(8):
                    nc.vector.tensor_scalar_mul(
                        v_all[:, c, :, 0:DH], v_all[:, c, :, 0:DH],
                        rsqv[:, c:c + 1])

                # --- q for own chunk, 4 head-groups of 4 ---
                def q_transposes(ng, qf):
                    for hh in range(4):
                        pt = pstr.tile([128, 128], F16, tag="pt")
                        nc.tensor.transpose(pt, qf[:, hh, :], ident16)
                        nc.vector.tensor_copy(qT_all[:, ng * 4 + hh, :], pt)

                pend_q = None
                for ng in range(4):
                    wqs = att.tile([128, KH, 512], F16, tag="wst")
                    nc.scalar.dma_start(
                        out=wqs,
                        in_=bass.AP(tensor=wq_in,
                                    offset=ng * KH * 128 * 512,
                                    ap=[[512, 128], [128 * 512, KH],
                                        [1, 512]]))
                    pq = psB.tile([TC, 512], F32, tag="p512")
                    for j in range(KH):
                        nc.tensor.matmul(
                            pq, xTq_sb[:, j, :], wqs[:, j, :],
                            start=(j == 0), stop=(j == KH - 1))
                    qraw = att.tile([TC, 512], F32, tag="qraw")
                    nc.scalar.activation(qraw, pq, AF.Copy, scale=1.0)
                    q3 = qraw.rearrange("p (h d) -> p h d", h=4)
                    qk_norm(q3, 4, "q")
                    qf = att.tile([TC, 4, DH], F16, tag="qf")
                    rope(q3, qf, 4, rope_q_sb)
                    if pend_q is not None:
                        q_transposes(*pend_q)
                    pend_q = (ng, qf)
                q_transposes(*pend_q)

                # --- per-head scores / ctx (1-deep software pipeline) ---
                def head_front(h):
                    g = h // (NH // NKV)
                    ps = psA.tile([128, 1024], F32, tag="p1k")
                    for c8 in range(8):
                        nc.tensor.matmul(
                            ps[:, c8 * TC:(c8 + 1) * TC],
                            kT_all[:, g, c8, :], qT_all[:, h, :],
                            start=True, stop=True)
                    mf = mask_sb.rearrange("p c q -> p (c q)")
                    nc.vector.tensor_add(ps, ps, mf)
                    pr = att.tile([128, 8, TC], F16, tag="probs")
                    nc.scalar.activation(
                        pr.rearrange("p c q -> p (c q)"), ps, AF.Exp,
                        bias=neg3, scale=SCALE)
                    return pr

                def tail_mm(h, pr):
                    g = h // (NH // NKV)
                    pct = psB.tile([TC, 512], F32, tag="p512")
                    pc = pct[:, 0:DH + 1]
                    for c8 in range(8):
                        nc.tensor.matmul(pc, pr[:, c8, :], v_all[:, c8, g, :],
                                         start=(c8 == 0), stop=(c8 == 7))
                    return pc

                def tail_fin(h, pc):
                    rden = att.tile([TC, 1], F32, tag="rden")
                    nc.vector.reciprocal(rden, pc[:, DH:DH + 1])
                    cf = att.tile([TC, DH], F16, tag="cf")
                    nc.vector.tensor_scalar_mul(cf, pc[:, 0:DH], rden)
                    pt = pstr.tile([128, 128], F16, tag="pt")
                    nc.tensor.transpose(pt, cf, ident16)
                    nc.scalar.activation(ctxT_all[:, h, :], pt,
                                         AF.Copy, scale=1.0)

                pend_mm = None
                pend_fin = None
                for h in range(NH):
                    pr = head_front(h)
                    if pend_mm is not None:
                        pc = tail_mm(*pend_mm)
                        if pend_fin is not None:
                            tail_fin(*pend_fin)
                        pend_fin = (pend_mm[0], pc)
                    pend_mm = (h, pr)
                pc = tail_mm(*pend_mm)
                tail_fin(*pend_fin)
                tail_fin(pend_mm[0], pc)

                # --- wo + residual + rms stats ---
                for n in range(4):
                    wos = att.tile([128, KH, 512], F16, tag="wst")
                    nc.sync.dma_start(
                        out=wos,
                        in_=bass.AP(tensor=wo_in,
                                    offset=n * KH * 128 * 512,
                                    ap=[[512, 128], [128 * 512, KH],
                                        [1, 512]]))
                    po = psB.tile([TC, 512], F32, tag="p512")
                    for j in range(KH):
                        nc.tensor.matmul(
                            po, ctxT_all[:, j, :], wos[:, j, :],
                            start=(j == 0), stop=(j == KH - 1))
                    at16 = att.tile([TC, 512], F16, tag="at16")
                    nc.scalar.activation(at16, po, AF.Copy, scale=1.0)
                    sl = slice(n * 512, (n + 1) * 512)
                    nc.vector.tensor_add(x2_sb[:, sl], at16, x_sb[:, sl])
                    sqp = att.tile([TC, 512], F32, tag="sqp")
                    nc.gpsimd.tensor_mul(sqp, x2_sb[:, sl], x2_sb[:, sl])
                    nc.vector.tensor_reduce(red4[:, n:n + 1], sqp, axis=AX.X,
                                            op=ALU.add)

                # rs2 true + rs2*AS_H
                rs2t = att.tile([TC, 1], F32, tag="rs2t", bufs=1)
                nc.vector.tensor_reduce(rs2t, red4, axis=AX.X, op=ALU.add)
                nc.scalar.activation(rs2t, rs2t, AF.Sqrt, bias=eps_sb[:TC],
                                     scale=1.0 / H)
                nc.vector.reciprocal(rs2t, rs2t)
                rs2h = att.tile([TC, 1], F32, tag="rs2h", bufs=1)
                nc.scalar.activation(rs2h, rs2t, AF.Copy, scale=AS_H)
                nc.vector.tensor_scalar_mul(h2_f16, x2_sb, rs2h)

                # --- router (fp32) interleaved with h8/r8 planes:
                # per j: PE does tr32(j), tr16(j), router-mm(j-1); Act/DVE
                # drain the psums. Keeps every engine fed, AG starts ASAP.
                prlt = psA.tile([128, 1024], F32, tag="p1k")
                prl = prlt[:, 0:E]
                pend_r = None
                for j in range(KH):
                    ptft = psB.tile([TC, 512], F32, tag="p512")
                    ptf = ptft[:, 0:128]
                    nc.tensor.transpose(
                        ptf, x2_sb[:, j * 128:(j + 1) * 128], ident32)
                    pt = pstr.tile([128, 128], F16, tag="pt")
                    nc.tensor.transpose(
                        pt, h2_f16[:, j * 128:(j + 1) * 128], ident16)
                    if pend_r is not None:
                        nc.tensor.matmul(prl, pend_r, wrT_sb[:, j - 1, :],
                                         start=(j == 1), stop=False)
                    x2T = att.tile([128, TC], F32, tag="x2T")
                    nc.vector.tensor_copy(x2T, ptf)
                    pend_r = x2T
                    nc.scalar.activation(h8_sb[:, j, :], pt, AF.Copy,
                                         scale=1.0)
                    rr16 = att.tile([128, TC], F16, tag="rr16")
                    nc.vector.tensor_sub(rr16, pt, h8_sb[:, j, :])
                    nc.scalar.activation(r8_sb[:, j, :], rr16, AF.Copy,
                                         scale=1.0)
                nc.tensor.matmul(prl, pend_r, wrT_sb[:, KH - 1, :],
                                 start=False, stop=True)
                nc.sync.dma_start(
                    out=bass.AP(tensor=ag_in.tensor, offset=ag_in.offset,
                                ap=[[TC, 128], [128 * TC, KH], [1, TC]]),
                    in_=h8_sb)
                nc.scalar.dma_start(
                    out=bass.AP(tensor=ag_in.tensor,
                                offset=ag_in.offset + H8SZ,
                                ap=[[TC, 128], [128 * TC, KH], [1, TC]]),
                    in_=r8_sb)
                lg = att.tile([TC, E], F32, tag="lg", bufs=1)
                nc.vector.tensor_scalar_mul(lg, prl, rs2t)
                probs8 = att.tile([TC, E], F32, tag="probs8", bufs=1)
                nc.scalar.activation(probs8, lg, AF.Exp, scale=1.0)
                den8 = att.tile([TC, 1], F32, tag="den8", bufs=1)
                nc.vector.tensor_reduce(den8, probs8, axis=AX.X, op=ALU.add)
                nc.vector.reciprocal(den8, den8)
                nc.vector.tensor_scalar_mul(probs8, probs8, den8)
                mx8 = att.tile([TC, 8], F32, tag="mx8", bufs=1)
                nc.vector.max(out=mx8, in_=probs8)
                s12 = att.tile([TC, 1], F32, tag="s12", bufs=1)
                nc.vector.tensor_add(s12, mx8[:, 0:1], mx8[:, 1:2])
                nc.vector.reciprocal(s12, s12)
                eq1 = att.tile([TC, E], F32, tag="eq1", bufs=1)
                nc.vector.tensor_scalar(eq1, probs8, mx8[:, 0:1], None,
                                        op0=ALU.is_equal)
                eq2 = att.tile([TC, E], F32, tag="eq2", bufs=1)
                nc.vector.tensor_scalar(eq2, probs8, mx8[:, 1:2], None,
                                        op0=ALU.is_equal)
                nc.vector.tensor_add(eq1, eq1, eq2)
                wm = att.tile([TC, E], F32, tag="wm", bufs=1)
                nc.vector.tensor_mul(wm, probs8, eq1)
                nc.vector.tensor_scalar_mul(wm, wm, s12)
                nc.gpsimd.dma_start(
                    out=bass.AP(tensor=ag_in.tensor,
                                offset=ag_in.offset + WOFF,
                                ap=[[E * 4, TC], [1, E * 4]]),
                    in_=wm.bitcast(F8))
                nc.sync.dma_start(out=dbg_wm[:, :], in_=wm)
                nc.scalar.dma_start(out=dbg_x2[:, :], in_=x2_sb)


            nc.gpsimd.collective_compute(
                "AllGather", ALU.bypass, replica_groups=RG_PAIR,
                ins=[ag_in.opt()], outs=[ag_out.opt()])

            # ================= MOE =================
            with tc.tile_pool(name="moe", bufs=2) as moe, \
                 tc.tile_pool(name="wgup", bufs=3) as wgup:

                # weights loaded in the AG window (DMA is idle there)
                wsgu_sb = moe.tile([128, KH, 2 * ISH], F16, tag="wsgu",
                                   bufs=1)
                nc.scalar.dma_start(
                    out=wsgu_sb,
                    in_=bass.AP(tensor=wsgu, offset=0,
                                ap=[[2 * ISH, 128], [128 * 2 * ISH, KH],
                                    [1, 2 * ISH]]))
                wsd_sb = moe.tile([128, ISH // 128, H], F16, tag="wsd",
                                  bufs=1)
                nc.sync.dma_start(
                    out=wsd_sb,
                    in_=bass.AP(tensor=wsd, offset=0,
                                ap=[[H, 128], [128 * H, ISH // 128],
                                    [1, H]]))
                wd_sb = moe.tile([128, EH * 4, 2, H], F8, tag="wd", bufs=1)
                for s in range(EH):
                    (nc.sync if s % 2 == 0 else nc.scalar).dma_start(
                        out=wd_sb[:, s * 4:(s + 1) * 4, :, :],
                        in_=bass.AP(tensor=wd_in, offset=s * 4 * 128 * 2 * H,
                                    ap=[[2 * H, 128], [128 * 2 * H, 4],
                                        [H, 2], [1, H]]))

                # gathered h8/r8 pair planes [128, jj, pl, tok(rank-major)]
                h8P = moe.tile([128, KJ, 2, TP], F8, tag="h8P", bufs=1)
                r8P = moe.tile([128, KJ, 2, TP], F8, tag="r8P", bufs=1)
                for r in range(2):
                    base = ag_out.offset + r * AGSZ
                    nc.gpsimd.dma_start(
                        out=h8P[:, :, :, r * TC:(r + 1) * TC],
                        in_=bass.AP(tensor=ag_out.tensor, offset=base,
                                    ap=[[TC, 128], [2 * 128 * TC, KJ],
                                        [128 * TC, 2], [1, TC]]))
                    nc.gpsimd.dma_start(
                        out=r8P[:, :, :, r * TC:(r + 1) * TC],
                        in_=bass.AP(tensor=ag_out.tensor, offset=base + H8SZ,
                                    ap=[[TC, 128], [2 * 128 * TC, KJ],
                                        [128 * TC, 2], [1, TC]]))
                # router weights of the pair
                wpair = moe.tile([128, 2, E], F32, tag="wpair", bufs=1)
                for r in range(2):
                    nc.gpsimd.dma_start(
                        out=wpair[:, r, :],
                        in_=bass.AP(tensor=ag_out.tensor,
                                    offset=ag_out.offset + r * AGSZ + WOFF,
                                    ap=[[E * 4, TC], [1, E * 4]]).bitcast(F32))

                # reconstruct fp16 h2 pair (for shared expert)
                h2P16 = moe.tile([128, KJ, 2, TP], F16, tag="h2P16", bufs=1)
                nc.vector.tensor_add(
                    h2P16.rearrange("p a b t -> p (a b t)"),
                    h8P.rearrange("p a b t -> p (a b t)"),
                    r8P.rearrange("p a b t -> p (a b t)"))

                # per-expert token weight rows -> broadcast tiles
                wcols = moe.tile([128, 2, EH], F32, tag="wcols", bufs=1)
                for s in range(EH):
                    wtmp = moe.tile([128, 2, E], F32, tag="wtmp")
                    nc.vector.tensor_mul(
                        wtmp, wpair,
                        _bc(esel_sb[:, s * E:(s + 1) * E], 2))
                    nc.vector.tensor_reduce(wcols[:, :, s:s + 1], wtmp,
                                            axis=AX.X, op=ALU.add)
                for r in range(2):
                    nc.gpsimd.dma_start(
                        out=bass.AP(tensor=wrow_dram.tensor,
                                    offset=wrow_dram.offset + r * TC,
                                    ap=[[1, 128], [TP, EH]]),
                        in_=wcols[:, r, :])
                w_bcast = moe.tile([128, EH, TP], F32, tag="w_bcast", bufs=1)
                for s in range(EH):
                    nc.gpsimd.dma_start(
                        out=w_bcast[:, s, :],
                        in_=bass.AP(tensor=wrow_dram.tensor,
                                    offset=wrow_dram.offset + s * TP,
                                    ap=[[0, 128], [1, TP]]))

                # --- shared expert (fp16, half IS, pair tokens) ---
                actsT_sh = moe.tile([128, ISH // 128, TP], F16, tag="actsTsh",
                                    bufs=1)
                for ch in range(2):
                    psg = psA.tile([128, 1024], F32, tag="p1k")
                    for hf in range(2):
                        for j16 in range(KH):
                            nc.tensor.matmul(
                                psg[:, hf * 512:(hf + 1) * 512],
                                h2P16[:, j16 // 2, j16 % 2,
                                      ch * TC:(ch + 1) * TC],
                                wsgu_sb[:, j16, hf * 512:(hf + 1) * 512],
                                start=(j16 == 0), stop=(j16 == KH - 1))
                    gsh = moe.tile([128, ISH], F16, tag="gsh")
                    nc.scalar.activation(gsh, psg[:, 0:ISH], AF.Silu,
                                         scale=1.0 / AS_H)
                    ush = moe.tile([128, ISH], F16, tag="ush")
                    nc.scalar.activation(ush, psg[:, ISH:2 * ISH], AF.Copy,
                                         scale=1.0 / AS_H)
                    acts = moe.tile([128, ISH], F16, tag="acts")
                    nc.vector.tensor_mul(acts, gsh, ush)
                    for it in range(ISH // 128):
                        pt = pstr.tile([128, 128], F16, tag="pt")
                        nc.tensor.transpose(
                            pt, acts[:, it * 128:(it + 1) * 128], ident16)
                        if it % 2 == 0:
                            nc.vector.tensor_copy(
                                actsT_sh[:, it, ch * TC:(ch + 1) * TC], pt)
                        else:
                            nc.scalar.activation(
                                actsT_sh[:, it, ch * TC:(ch + 1) * TC], pt,
                                AF.Copy)

                # --- routed experts: gu (fp8 DR, h8+r8 K-chain) ---
                act_pr = moe.tile([128, EH * 4, 2, TP], F8, tag="act_pr",
                                  bufs=1)
                for s in range(EH):
                    g16 = moe.tile([128, 8, TP], F16, tag="g16")
                    wkc = None
                    for mi in range(16):
                        if mi % 2 == 0:
                            wkc = wgup.tile([128, 2, KJ, 2, 128], F8,
                                            tag="wgu")
                            (nc.sync if mi % 4 == 0 else nc.scalar).dma_start(
                                out=wkc,
                                in_=bass.AP(
                                    tensor=wgu_in,
                                    offset=(s * 16 + mi) * 128 * 2048,
                                    ap=[[2048, 128], [128 * 2048, 2],
                                        [256, KJ], [128, 2], [1, 128]]))
                        pgt = psB.tile([TC, 512], F32, tag="p512")
                        pg = pgt[:, 0:TP]
                        for jj in range(KJ):
                            nc.tensor.matmul(
                                pg, wkc[:, mi % 2, jj, :, :],
                                h8P[:, jj, :, :],
                                start=(jj == 0), stop=False,
                                perf_mode=PM.DoubleRow)
                        for jj in range(KJ):
                            nc.tensor.matmul(
                                pg, wkc[:, mi % 2, jj, :, :],
                                r8P[:, jj, :, :],
                                start=False, stop=(jj == KJ - 1),
                                perf_mode=PM.DoubleRow)
                        if mi < 8:
                            nc.scalar.activation(g16[:, mi, :], pg, AF.Silu,
                                                 scale=SILU_SC)
                        else:
                            iu = mi - 8
                            u16 = moe.tile([128, TP], F16, tag="u16", bufs=3)
                            nc.vector.tensor_mul(u16, pg, w_bcast[:, s, :])
                            (nc.vector if iu % 2 == 0
                             else nc.gpsimd).tensor_mul(
                                act_pr[:, s * 4 + iu // 2, iu % 2, :],
                                g16[:, iu, :], u16)

                # --- down proj (routed fp8 DR + shared fp16 in one psum) ---
                for ch in range(2):
                    for n in range(4):
                        pd = psB.tile([TC, 512], F32, tag="p512")
                        for kk in range(EH * 4):
                            nc.tensor.matmul(
                                pd,
                                act_pr[:, kk, :, ch * TC:(ch + 1) * TC],
                                wd_sb[:, kk, :, n * 512:(n + 1) * 512],
                                start=(kk == 0), stop=False,
                                perf_mode=PM.DoubleRow)
                        for it in range(ISH // 128):
                            nc.tensor.matmul(
                                pd,
                                actsT_sh[:, it, ch * TC:(ch + 1) * TC],
                                wsd_sb[:, it, n * 512:(n + 1) * 512],
                                start=False, stop=(it == ISH // 128 - 1))
                        rsd = moe.tile([TC, 512], F16, tag="rsd", bufs=3)
                        nc.scalar.activation(rsd, pd, AF.Copy, scale=DQ_D)
                        (nc.sync if (ch * 4 + n) % 2 == 0
                         else nc.scalar).dma_start(
                            out=rs_in[ch * TC:(ch + 1) * TC,
                                      n * 512:(n + 1) * 512],
                            in_=rsd)

                nc.gpsimd.collective_compute(
                    "ReduceScatter", ALU.add, replica_groups=RG_PAIR,
                    ins=[rs_in.opt()], outs=[rs_out.opt()])

                # --- combine + residual ---
                for q in range(4):
                    sl = slice(q * 512, (q + 1) * 512)
                    rsld = moe.tile([TC, 512], F16, tag="rsld")
                    (nc.sync if q % 2 == 0 else nc.scalar).dma_start(
                        out=rsld, in_=rs_out[:, sl])
                    outf = moe.tile([TC, 512], F32, tag="outf")
                    nc.vector.tensor_add(outf, x2_sb[:, sl], rsld)
                    (nc.sync if q % 2 == 0 else nc.scalar).dma_start(
                        out=out_chunk[:, sl], in_=outf)
                    nc.scalar.dma_start(out=dbg_moe[:, sl], in_=rsld)

    nc.compile()
    return nc


def _prep_inputs(hidden_states, w_ln1, w_ln2, wqkv, q_norm_w, k_norm_w, wo,
                 w_router, w_gu, w_d, ws_gu, ws_d, positions):
    import ml_dtypes
    f16 = np.float16
    f8 = ml_dtypes.float8_e4m3

    x = np.asarray(hidden_states, np.float32).reshape(T, H)
    w_ln1 = np.asarray(w_ln1, np.float32)
    w_ln2 = np.asarray(w_ln2, np.float32)
    wqkv_e = np.asarray(wqkv, np.float32) * w_ln1[:, None]  # [H, 3072]

    def pack_dr(W, scale):
        # W [Hrows, C] -> [KJ, 128, 2, C]; row = 256*jj + 128*pl + r
        Wr = (np.asarray(W, np.float32) * scale).reshape(KJ, 2, 128, -1)
        return np.ascontiguousarray(Wr.transpose(0, 2, 1, 3)).astype(f8)

    def pack16(W):
        # W [Hrows, C] -> [KH, 128, C] fp16
        return np.ascontiguousarray(
            np.asarray(W, np.float32).reshape(KH, 128, -1)).astype(f16)

    def by_coltile16(W):
        # W [Hrows, C] -> [C//512, KH, 128, 512] fp16
        C = W.shape[1]
        return np.ascontiguousarray(
            np.asarray(W, np.float32).reshape(KH, 128, C // 512, 512)
            .transpose(2, 0, 1, 3)).astype(f16)

    wq_p = by_coltile16(wqkv_e[:, :NH * DH])
    wkv_p = pack16(wqkv_e[:, NH * DH:])
    xT_p = pack16(x.T)
    wo_p = by_coltile16(np.asarray(wo, np.float32))

    pos = np.asarray(positions).astype(np.float64)
    inv_freq = 1.0 / (10000.0 ** (np.arange(0, DH, 2, dtype=np.float64) / DH))
    freqs = pos[:, None] * inv_freq[None, :]
    cos = np.cos(freqs).astype(np.float32)
    sin = np.sin(freqs).astype(np.float32)
    qw = np.asarray(q_norm_w, np.float32)
    kw = np.asarray(k_norm_w, np.float32)

    def rope_tab(w):
        return np.ascontiguousarray(
            np.stack([cos * w[None, :64], sin * w[None, 64:],
                      cos * w[None, 64:], sin * w[None, :64]],
                     axis=1)).astype(f16)

    rq = rope_tab(qw)  # [T, 4, 64]
    rk = rope_tab(kw).reshape(8, TC, 4, 64)

    wrT_e = (np.asarray(w_router, np.float32) * w_ln2[None, :]).T  # [H, E]
    wrT_p = np.ascontiguousarray(
        wrT_e.reshape(KH, 128, E).transpose(1, 0, 2)).astype(np.float32)

    ws_gu_e = np.asarray(ws_gu, np.float32) * w_ln2[:, None]
    ws_d_e = np.asarray(ws_d, np.float32) * SHD_SC
    w_gu_e = np.asarray(w_gu, np.float32) * w_ln2[None, :, None] * WS_GU
    w_d_e = np.asarray(w_d, np.float32) * WS_D

    kidx = np.arange(T)
    in_maps = []
    for c in range(N_CORES):
        rows = np.arange(c * TC, (c + 1) * TC)
        # mask[p, kc, q]: k token = kc*128+p, q token = c*128+q
        kk = kidx.reshape(8, 128)
        mask = np.where(kk.T[:, :, None] <= rows[None, None, :], 0.0, NEG)
        mask = np.ascontiguousarray(mask).astype(f16)

        ph = c % 2
        # shared half: own g/u columns
        wsgu_half = np.concatenate(
            [ws_gu_e[:, ph * ISH:(ph + 1) * ISH],
             ws_gu_e[:, IS + ph * ISH:IS + (ph + 1) * ISH]], axis=1)
        wsgu_p = np.ascontiguousarray(
            wsgu_half.reshape(KH, 128, 2 * ISH)).astype(f16)
        wsd_p = np.ascontiguousarray(
            ws_d_e[ph * ISH:(ph + 1) * ISH].reshape(ISH // 128, 128, H)
        ).astype(f16)

        # routed experts for this half
        es = np.zeros((1, EH * E), np.float32)
        wgu_p = np.empty((EH, 16, 128, KJ, 2, 128), f8)
        wd_p = np.empty((EH, 4, 128, 2, H), f8)
        for s in range(EH):
            e = ph * EH + s
            es[0, s * E + e] = U_SC
            wg = w_gu_e[e].reshape(KJ, 2, 128, 16, 128)  # [jj, pl, r, mi, m]
            wgu_p[s] = wg.transpose(3, 2, 0, 1, 4).astype(f8)
            wdv = w_d_e[e].reshape(4, 2, 128, H)  # [j, pl, r, h]
            wd_p[s] = wdv.transpose(0, 2, 1, 3).astype(f8)

        in_maps.append({
            "x_res": np.ascontiguousarray(x[c * TC:(c + 1) * TC]),
            "xT_in": xT_p,
            "xTq_in": np.ascontiguousarray(
                xT_p[:, :, c * TC:(c + 1) * TC]),
            "wkv_in": wkv_p,
            "wq_in": wq_p,
            "wo_in": wo_p,
            "rope_k": rk,
            "rope_q": np.ascontiguousarray(rq[c * TC:(c + 1) * TC]),
            "mask_in": mask,
            "wrT": wrT_p,
            "wsgu": wsgu_p,
            "wsd": wsd_p,
            "wgu_in": np.ascontiguousarray(wgu_p),
            "wd_in": np.ascontiguousarray(wd_p),
            "esel": es,
        })
    return in_maps


def kernel(**inputs):
    import os
    if "nc" not in _cache:
        _cache["nc"] = build_nc()
    nc = _cache["nc"]
    in_maps = _prep_inputs(**inputs)
    trace = bool(int(os.environ.get("KERNEL_TRACE", "0")))
    res = run_bass_kernel_spmd(nc, in_maps, core_ids=list(range(N_CORES)),
                               trace=trace)
    _cache["last_result"] = res
    out = np.concatenate(
        [res.results[c]["out_chunk"] for c in range(N_CORES)], axis=0)
    return out.reshape(1, T, H).astype(np.float32)


if __name__ == "__main__":
    import reference
    inp = {k: np.asarray(v) for k, v in reference.setup_inputs().items()}
    got = kernel(**inp)
    exp = np.asarray(reference.reference(**reference.setup_inputs()))
    denom = np.abs(exp).max()
    err = np.abs(got - exp).max() / denom
    print("abs max:", denom, "rel err:", err)
